# revision 1
# baseline (speedup 1.0000x reference)
"""MLA prefill kernel for Trainium2, 8 NeuronCores.

Sharding: core c -> (batch b = c // 2, head-group g = c % 2). Each core
computes its batch's full sequence for its 8 heads, producing a partial
output (transposed, [2048, 1024]); the host sums the two head-group
partials per batch and transposes back.

Layout strategy (all on-chip matmuls contract over the partition dim):
  x is passed in transposed ([D, L]) per batch.  Down/up projections
  produce latent-major / headdim-major activations directly.  Attention
  runs "k-major": scores^T [k_tok, q_tok] tiles, exp on ACT, denominators
  via ones-matmul column sums, value aggregation (LO^T) needs no P
  transposes.  Softmax max-subtraction is skipped (scores are O(1);
  mathematically identical).  Causality: strictly-upper key blocks are
  skipped (exp underflows to exactly 0 in the reference), diagonal blocks
  masked with affine_select after exp.
"""

import math
import os
from contextlib import ExitStack

import numpy as np

import concourse.bass as bass
import concourse.mybir as mybir
import concourse.tile as tile
from concourse import bacc, bass_utils
from concourse.masks import make_identity

# ---- problem constants -------------------------------------------------
B, L, D = 4, 1024, 2048
H, NOPE, ROPE, VD, KVR = 16, 128, 64, 128, 512
DQ = 1024            # q latent dim
HD = NOPE + ROPE     # 192 per-head q dim
EPS = 1e-6
NH = 8               # heads per core
N_CORES = 8
SCALE = 1.0 / math.sqrt(NOPE + ROPE)

F32 = mybir.dt.float32
F32R = mybir.dt.float32r

USE_F32R = os.environ.get("MLA_F32", "0") != "1"
DT = F32R if USE_F32R else F32
REPS = int(os.environ.get("MLA_REPS", "1"))  # timing amplification only

TOK = 512            # q-token tile (free dim of most matmuls)
NTOK = L // TOK      # 2
KB = 128             # key-token block
NKB = L // KB        # 8
ND = D // 128        # 16 contraction blocks over model dim
NLAT = DQ // 128     # 8 contraction blocks over q latent
NKV = KVR // 128     # 4 blocks over kv latent


def build_nc():
    nc = bacc.Bacc("TRN2", target_bir_lowering=False, debug=False)

    t = {}
    t["x_t"] = nc.dram_tensor("x_t", [D, L], DT, kind="ExternalInput").ap()
    t["wqd_t"] = nc.dram_tensor("wqd_t", [D, DQ], DT, kind="ExternalInput").ap()
    t["wqu_t"] = nc.dram_tensor("wqu_t", [DQ, NH * HD], DT, kind="ExternalInput").ap()
    t["wkvd_t"] = nc.dram_tensor(
        "wkvd_t", [D, KVR + ROPE], DT, kind="ExternalInput"
    ).ap()
    t["wupn"] = nc.dram_tensor("wupn", [NH, NOPE, KVR], DT, kind="ExternalInput").ap()
    t["wupv_t"] = nc.dram_tensor(
        "wupv_t", [NH, 128, NKV, VD], DT, kind="ExternalInput"
    ).ap()
    t["wout_t"] = nc.dram_tensor(
        "wout_t", [NH * VD, D], DT, kind="ExternalInput"
    ).ap()
    t["cosf"] = nc.dram_tensor("cosf", [128, L], F32, kind="ExternalInput").ap()
    t["sinf"] = nc.dram_tensor("sinf", [128, L], F32, kind="ExternalInput").ap()
    t["vn_d"] = nc.dram_tensor("vn_d", [NH, VD, L], DT, kind="Internal").ap()
    t["outT"] = nc.dram_tensor("outT", [D, L], F32, kind="ExternalOutput").ap()

    with tile.TileContext(nc) as tc:
        _emit(tc, t)
    nc.compile()
    return nc


def _emit(tc, t):
    nc = tc.nc
    with ExitStack() as c0:
        c0.enter_context(
            nc.allow_low_precision(reason="fp32r rounding is intentional")
        )
        glob = c0.enter_context(tc.tile_pool(name="glob", bufs=1))
        ps_a = c0.enter_context(tc.tile_pool(name="ps_a", bufs=2, space="PSUM"))
        ps_lo = c0.enter_context(tc.tile_pool(name="ps_lo", bufs=4, space="PSUM"))
        ps_dv = c0.enter_context(tc.tile_pool(name="ps_dv", bufs=2, space="PSUM"))

        # ---- constants ------------------------------------------------
        from concourse import library_config

        nc.gpsimd.load_library(library_config.attnmlp)
        ident = glob.tile([128, 128], F32, tag="ident")
        make_identity(nc, ident)
        ones_f32 = glob.tile([128, 128], F32, tag="ones32")
        nc.vector.memset(ones_f32, 1.0)
        ones_col = glob.tile([128, 1], DT, tag="ones")
        nc.vector.tensor_copy(ones_col, ones_f32[:, :1])
        ones_row = glob.tile([1, 128], DT, tag="onesr")
        nc.vector.tensor_copy(ones_row, ones_f32[:1, :])
        eps_t = glob.tile([1, 1], F32, tag="eps")
        nc.vector.memset(eps_t, EPS)
        cosf = glob.tile([128, L], F32, tag="cosf")
        nc.sync.dma_start(out=cosf, in_=t["cosf"])
        sinf = glob.tile([128, L], F32, tag="sinf")
        nc.sync.dma_start(out=sinf, in_=t["sinf"])

        for _rep in range(REPS):
            with ExitStack() as c1:
                p1 = c1.enter_context(tc.tile_pool(name=f"p1_{_rep}", bufs=1))
                kv_lat = p1.tile([128, NKV, L], DT, tag="kvlat")
                k_roped = p1.tile([128, L], DT, tag="kroped")

                with ExitStack() as c2:
                    p2 = c2.enter_context(
                        tc.tile_pool(name=f"p2_{_rep}", bufs=1)
                    )
                    qT_nope = p2.tile([128, NH, L], DT, tag="qnope")
                    q_roped = p2.tile([128, NH // 2, L], DT, tag="qroped")

                    _emit_front(tc, t, glob, ps_a, ps_dv,
                                ident, ones_col, eps_t, cosf, sinf,
                                kv_lat, k_roped, qT_nope, q_roped, _rep)

                    _emit_attn(tc, t, glob, ps_a, ps_lo, ps_dv,
                               ident, ones_col, ones_row,
                               kv_lat, k_roped, qT_nope, q_roped, _rep)

            _emit_outproj(tc, t, glob, ps_a, _rep)


def _emit_front(tc, t, glob, ps_a, ps_dv, ident, ones_col, eps_t, cosf, sinf,
                kv_lat, k_roped, qT_nope, q_roped, rep=0):
    """Down projections, RMS norms, k-rope, q up-projection + q-rope."""
    nc = tc.nc
    with ExitStack() as c3:
        p3 = c3.enter_context(tc.tile_pool(name=f"p3_{rep}", bufs=1))
        p3s = c3.enter_context(tc.tile_pool(name=f"p3s_{rep}", bufs=3))
        q_lat = p3.tile([128, NLAT, L], DT, tag="qlat")
        kr_pair = p3.tile([128, 2, L], F32, tag="krpair")

        # ---- phase 1: down projections, x streamed in d-halves -------
        with ExitStack() as c4:
            p4 = c4.enter_context(tc.tile_pool(name=f"p4_{rep}", bufs=1))
            p4s = c4.enter_context(tc.tile_pool(name=f"p4s_{rep}", bufs=2))
            wqd_r = t["wqd_t"].rearrange("(b p) m -> p b m", p=128)
            wkvd_r = t["wkvd_t"].rearrange("(b p) m -> p b m", p=128)
            x_r = t["x_t"].rearrange("(b p) t -> p b t", p=128)

            for half in range(2):
                hs = slice(half * 8, half * 8 + 8)
                xh = p4.tile([128, 8, L], DT, tag="xh", bufs=1)
                nc.sync.dma_start(out=xh, in_=x_r[:, hs, :])

                for lb in range(NLAT):
                    wqd = p4s.tile([128, 8, 128], DT, tag="wqd")
                    nc.sync.dma_start(
                        out=wqd, in_=wqd_r[:, hs, lb * 128 : (lb + 1) * 128]
                    )
                    for tk in range(NTOK):
                        ts = slice(tk * TOK, (tk + 1) * TOK)
                        ps = ps_a.tile([128, TOK], F32, tag="a")
                        for db in range(8):
                            nc.tensor.matmul(
                                ps, wqd[:, db, :], xh[:, db, ts],
                                start=(db == 0), stop=(db == 7),
                            )
                        dst = q_lat[:, lb, ts]
                        if half == 0:
                            nc.vector.tensor_copy(dst, ps)
                        else:
                            nc.vector.tensor_add(dst, dst, ps)

                for mb in range(NKV + 1):
                    mw = 128 if mb < NKV else ROPE
                    wkv = p4s.tile([128, 8, 128], DT, tag="wkv")
                    nc.sync.dma_start(
                        out=wkv[:, :, :mw],
                        in_=wkvd_r[:, hs, mb * 128 : mb * 128 + mw],
                    )
                    for tk in range(NTOK):
                        ts = slice(tk * TOK, (tk + 1) * TOK)
                        ps = ps_a.tile([128, TOK], F32, tag="a")
                        for db in range(8):
                            nc.tensor.matmul(
                                ps[:mw], wkv[:, db, :mw], xh[:, db, ts],
                                start=(db == 0), stop=(db == 7),
                            )
                        if mb < NKV:
                            dst = kv_lat[:, mb, ts]
                        else:
                            dst = kr_pair[:ROPE, 0, ts]
                        if half == 0:
                            nc.vector.tensor_copy(dst, ps[:mw])
                        else:
                            nc.vector.tensor_add(dst, dst, ps[:mw])

        # ---- phase 1.5: RMS-normalize q_lat (latent-major) -----------
        rq_row = p3.tile([1, L], F32, tag="rqrow")
        for tk in range(NTOK):
            ts = slice(tk * TOK, (tk + 1) * TOK)
            ps_ssq = ps_dv.tile([1, TOK], F32, tag="dv")
            for lb in range(NLAT):
                sq = p3s.tile([128, TOK], DT, tag="sq")
                sl = q_lat[:, lb, ts]
                nc.vector.tensor_mul(sq, sl, sl)
                nc.tensor.matmul(
                    ps_ssq, ones_col, sq,
                    start=(lb == 0), stop=(lb == NLAT - 1),
                )
            rt = p3s.tile([1, TOK], F32, tag="rt")
            nc.scalar.activation(
                rt, ps_ssq, mybir.ActivationFunctionType.Sqrt,
                bias=eps_t, scale=1.0 / DQ,
            )
            nc.vector.reciprocal(rq_row[:, ts], rt)
        rq_b = p3.tile([128, L], F32, tag="rqb")
        nc.gpsimd.partition_broadcast(rq_b, rq_row)
        for lb in range(NLAT):
            nc.vector.tensor_mul(q_lat[:, lb, :], q_lat[:, lb, :], rq_b)

        # ---- phase 1.6: RMS-normalize kv_lat (latent-major) ----------
        rkv_row = p3.tile([1, L], F32, tag="rkvrow")
        for tk in range(NTOK):
            ts = slice(tk * TOK, (tk + 1) * TOK)
            ps_ssq = ps_dv.tile([1, TOK], F32, tag="dv")
            for lb in range(NKV):
                sq = p3s.tile([128, TOK], DT, tag="sq")
                sl = kv_lat[:, lb, ts]
                nc.vector.tensor_mul(sq, sl, sl)
                nc.tensor.matmul(
                    ps_ssq, ones_col, sq,
                    start=(lb == 0), stop=(lb == NKV - 1),
                )
            rt = p3s.tile([1, TOK], F32, tag="rt")
            nc.scalar.activation(
                rt, ps_ssq, mybir.ActivationFunctionType.Sqrt,
                bias=eps_t, scale=1.0 / KVR,
            )
            nc.vector.reciprocal(rkv_row[:, ts], rt)
        rkv_b = p3.tile([128, L], F32, tag="rkvb")
        nc.gpsimd.partition_broadcast(rkv_b, rkv_row)
        for lb in range(NKV):
            nc.vector.tensor_mul(kv_lat[:, lb, :], kv_lat[:, lb, :], rkv_b)

        # k rope: swap + rope, duplicated into both partition halves
        _rope_pair(nc, kr_pair, cosf, sinf, k_roped, 0)
        nc.sync.dma_start(out=k_roped[ROPE:], in_=k_roped[:ROPE])

        # ---- phase 2: q up-projection + q rope -----------------------
        with ExitStack() as c5:
            p5s = c5.enter_context(tc.tile_pool(name=f"p5s_{rep}", bufs=2))
            wqu_r = t["wqu_t"].rearrange("(b p) m -> p b m", p=128)
            for h in range(NH):
                wqu = p5s.tile([128, NLAT, HD], DT, tag="wqu")
                nc.sync.dma_start(out=wqu, in_=wqu_r[:, :, h * HD : (h + 1) * HD])
                q_pair = p5s.tile([128, 2, L], F32, tag="pair")
                for tk in range(NTOK):
                    ts = slice(tk * TOK, (tk + 1) * TOK)
                    ps_n = ps_a.tile([128, TOK], F32, tag="a")
                    for lb in range(NLAT):
                        nc.tensor.matmul(
                            ps_n, wqu[:, lb, :NOPE], q_lat[:, lb, ts],
                            start=(lb == 0), stop=(lb == NLAT - 1),
                        )
                    nc.vector.tensor_copy(qT_nope[:, h, ts], ps_n)
                    ps_rp = ps_a.tile([128, TOK], F32, tag="a")
                    for lb in range(NLAT):
                        nc.tensor.matmul(
                            ps_rp[:ROPE], wqu[:, lb, NOPE:],
                            q_lat[:, lb, ts],
                            start=(lb == 0), stop=(lb == NLAT - 1),
                        )
                    nc.vector.tensor_copy(q_pair[:ROPE, 0, ts], ps_rp[:ROPE])
                _rope_pair(nc, q_pair, cosf, sinf, q_roped[:, h // 2, :], h % 2)


def _emit_attn(tc, t, glob, ps_a, ps_lo, ps_dv, ident, ones_col, ones_row,
               kv_lat, k_roped, qT_nope, q_roped, rep=0):
    """kv transpose, per-(head, q-tile) attention; v^T written to DRAM."""
    nc = tc.nc
    with ExitStack() as c6:
        p6 = c6.enter_context(tc.tile_pool(name=f"p6_{rep}", bufs=1))
        p6s = c6.enter_context(tc.tile_pool(name=f"p6s_{rep}", bufs=2))
        p6w = c6.enter_context(tc.tile_pool(name=f"p6w_{rep}", bufs=3))

        # transpose normalized kv_lat -> token-major
        kv_tok = p6.tile([128, NKB, KVR], DT, tag="kvtok")
        for kb in range(NKB):
            ps = ps_a.tile([128, KVR], F32, tag="a")
            for lb in range(NKV):
                nc.tensor.transpose(
                    ps[:, lb * 128 : (lb + 1) * 128],
                    kv_lat[:, lb, kb * 128 : (kb + 1) * 128].bitcast(F32),
                    ident,
                )
            nc.vector.tensor_copy(kv_tok[:, kb, :], ps)

        for h in range(NH):
            hb = (h % 2) * 64
            wn = p6s.tile([128, KVR], DT, tag="wupn")
            nc.sync.dma_start(out=wn, in_=t["wupn"][h])
            wv = p6s.tile([128, NKV, VD], DT, tag="wupv")
            nc.sync.dma_start(out=wv, in_=t["wupv_t"][h])

            q_abs = p6.tile([128, NKV, L], DT, tag="qabs", bufs=1)
            for mb in range(NKV):
                for tk in range(NTOK):
                    ts = slice(tk * TOK, (tk + 1) * TOK)
                    ps = ps_a.tile([128, TOK], F32, tag="a")
                    nc.tensor.matmul(
                        ps, wn[:, mb * 128 : (mb + 1) * 128],
                        qT_nope[:, h, ts],
                    )
                    nc.vector.tensor_copy(q_abs[:, mb, ts], ps)

            for tk in range(NTOK):
                ts = slice(tk * TOK, (tk + 1) * TOK)
                nkb = (tk + 1) * (TOK // KB)
                ps_d = ps_dv.tile([1, TOK], F32, tag="dv")
                ps_los = [
                    ps_lo.tile([128, TOK], F32, tag="lo", name=f"pslo{i}")
                    for i in range(NKV)
                ]
                for kb in range(nkb):
                    ks = slice(kb * 128, (kb + 1) * 128)
                    ps_s = ps_a.tile([128, TOK], F32, tag="a")
                    for lb in range(NKV):
                        nc.tensor.matmul(
                            ps_s, kv_lat[:, lb, ks], q_abs[:, lb, ts],
                            start=(lb == 0), stop=False,
                        )
                    nc.tensor.matmul(
                        ps_s, k_roped[hb : hb + ROPE, ks],
                        q_roped[hb : hb + ROPE, h // 2, ts],
                        start=False, stop=True,
                    )
                    e_t = p6w.tile([128, TOK], DT, tag="e")
                    nc.scalar.activation(
                        e_t, ps_s, mybir.ActivationFunctionType.Exp, scale=SCALE
                    )
                    if kb >= tk * (TOK // KB):
                        nc.gpsimd.affine_select(
                            out=e_t, in_=e_t,
                            pattern=[[1, TOK]],
                            compare_op=mybir.AluOpType.is_ge,
                            fill=0.0,
                            base=tk * TOK - kb * 128,
                            channel_multiplier=-1,
                        )
                    nc.tensor.matmul(
                        ps_d, ones_col, e_t,
                        start=(kb == 0), stop=(kb == nkb - 1),
                    )
                    for lb in range(NKV):
                        nc.tensor.matmul(
                            ps_los[lb],
                            kv_tok[:, kb, lb * 128 : (lb + 1) * 128],
                            e_t,
                            start=(kb == 0), stop=(kb == nkb - 1),
                        )
                rd = p6w.tile([1, TOK], DT, tag="rd")
                nc.vector.reciprocal(rd, ps_d)
                ps_b = ps_dv.tile([128, TOK], F32, tag="dv")
                nc.tensor.matmul(ps_b, ones_row, rd)
                rb_sb = p6w.tile([128, TOK], F32, tag="rb")
                nc.vector.tensor_copy(rb_sb, ps_b)
                lo_t = p6.tile([128, NKV, TOK], DT, tag="lot", bufs=1)
                for lb in range(NKV):
                    nc.vector.tensor_copy(lo_t[:, lb, :], ps_los[lb])
                ps_v = ps_dv.tile([128, TOK], F32, tag="dv")
                for lb in range(NKV):
                    nc.tensor.matmul(
                        ps_v, wv[:, lb, :], lo_t[:, lb, :],
                        start=(lb == 0), stop=(lb == NKV - 1),
                    )
                vn = p6w.tile([128, TOK], DT, tag="vn")
                nc.vector.tensor_mul(vn, ps_v, rb_sb)
                nc.sync.dma_start(out=t["vn_d"][h, :, ts], in_=vn)


def _emit_outproj(tc, t, glob, ps_a, rep=0):
    """out^T [D, L] = sum_h Wout_h^T-blocks @ v_norm_h^T, in two d-halves."""
    nc = tc.nc
    with ExitStack() as c7:
        p7 = c7.enter_context(tc.tile_pool(name=f"p7_{rep}", bufs=1))
        p7s = c7.enter_context(tc.tile_pool(name=f"p7s_{rep}", bufs=3))
        wout_r = t["wout_t"].rearrange("(b p) m -> p b m", p=128)

        vn_sb = p7.tile([128, NH, L], DT, tag="vnsb")
        for h in range(NH):
            nc.sync.dma_start(out=vn_sb[:, h, :], in_=t["vn_d"][h])

        for dh in range(2):
            wout = p7.tile([128, NH, D // 2], DT, tag="wout", bufs=2)
            nc.sync.dma_start(
                out=wout, in_=wout_r[:, :, dh * (D // 2) : (dh + 1) * (D // 2)]
            )
            for db in range(D // 256):
                for tk in range(NTOK):
                    ts = slice(tk * TOK, (tk + 1) * TOK)
                    ps = ps_a.tile([128, TOK], F32, tag="a")
                    for h in range(NH):
                        nc.tensor.matmul(
                            ps, wout[:, h, db * 128 : (db + 1) * 128],
                            vn_sb[:, h, ts],
                            start=(h == 0), stop=(h == NH - 1),
                        )
                    o_t = p7s.tile([128, TOK], F32, tag="o")
                    nc.vector.tensor_copy(o_t, ps)
                    row = dh * (D // 2) + db * 128
                    nc.sync.dma_start(out=t["outT"][row : row + 128, ts], in_=o_t)


def _rope_pair(nc, pair, cosf, sinf, out, half):
    """pair[:64,0,:] = v in split re/im layout (re rows 0..31, im rows
    32..63); fill pair[:64,1,:] with the partner rows, then rope into out
    rows [half*64, half*64+64)."""
    hb = half * 64
    nc.sync.dma_start(out=pair[0:32, 1, :], in_=pair[32:ROPE, 0, :])
    nc.sync.dma_start(out=pair[32:ROPE, 1, :], in_=pair[0:32, 0, :])
    if hb:
        nc.sync.dma_start(out=pair[hb : hb + ROPE, :, :], in_=pair[:ROPE, :, :])
    a = pair[hb : hb + ROPE, 0, :]
    b = pair[hb : hb + ROPE, 1, :]
    ob = out[hb : hb + ROPE]
    # out = a*cos + b*sinf'  (sign of the swap folded into sinf)
    nc.vector.tensor_mul(ob, a, cosf[hb : hb + ROPE])
    nc.vector.tensor_mul(pair[hb : hb + ROPE, 0, :], b, sinf[hb : hb + ROPE])
    nc.vector.tensor_add(ob, ob, pair[hb : hb + ROPE, 0, :])


# ======================================================================
# host side
# ======================================================================

_NC_CACHE = {}


def _get_nc():
    key = ("nc", USE_F32R)
    if key not in _NC_CACHE:
        _NC_CACHE[key] = build_nc()
    return _NC_CACHE[key]


def _prep_shared(inputs):
    wq_down = np.asarray(inputs["Wq_down"], np.float32)
    wq_up = np.asarray(inputs["Wq_up"], np.float32)
    wkv_down = np.asarray(inputs["Wkv_down"], np.float32)
    wkv_up = np.asarray(inputs["Wkv_up"], np.float32)
    wout = np.asarray(inputs["Wout"], np.float32)
    rms_q_w = np.asarray(inputs["rms_q_w"], np.float32)
    rms_kv_w = np.asarray(inputs["rms_kv_w"], np.float32)
    freq = np.asarray(inputs["freq_cis"], np.float32)  # [L, 32, 2]

    # split re/im layout for all rope dims: re parts first, then im parts
    rope_perm = np.concatenate(
        [np.arange(0, ROPE, 2), np.arange(1, ROPE, 2)]
    )  # [64]

    wqd_t = np.ascontiguousarray(wq_down.T)  # [D, DQ]
    wkv_down_p = wkv_down.copy()
    wkv_down_p[KVR:] = wkv_down[KVR:][rope_perm]
    wkvd_t = np.ascontiguousarray(wkv_down_p.T)  # [D, 576]

    # rope tables (dim-major, split re/im, duplicated partition halves)
    cos = freq[:, :, 0].T  # [32, L]
    sin = freq[:, :, 1].T
    cosf64 = np.vstack([cos, cos])  # [64, L]
    sinf64 = np.vstack([-sin, sin])
    cosf = np.ascontiguousarray(np.vstack([cosf64, cosf64]))  # [128, L]
    sinf = np.ascontiguousarray(np.vstack([sinf64, sinf64]))

    wq_up3 = (wq_up * rms_q_w[None, :]).reshape(H, HD, DQ)
    wq_up3 = np.concatenate(
        [wq_up3[:, :NOPE, :], wq_up3[:, NOPE:, :][:, rope_perm, :]], axis=1
    )
    wkv_up3 = wkv_up.reshape(H, NOPE + VD, KVR)
    wout3 = wout.reshape(D, H, VD)

    per_g = []
    for g in range(2):
        hs = list(range(g * NH, (g + 1) * NH))
        wqu_t = np.ascontiguousarray(
            wq_up3[hs].reshape(NH * HD, DQ).T
        )  # [DQ, 1536]
        wupn = np.ascontiguousarray(
            wkv_up3[hs, :NOPE, :] * rms_kv_w[None, None, :]
        )  # [8, 128, 512]
        wupv = wkv_up3[hs, NOPE:, :] * rms_kv_w[None, None, :]  # [8, 128, 512]
        # -> lhsT layout per head: [512, 128] -> [4, 128, 128] -> [128, 4, 128]
        wupv_t = np.ascontiguousarray(
            wupv.transpose(0, 2, 1).reshape(NH, NKV, 128, VD).transpose(0, 2, 1, 3)
        )  # [8, 128, 4, 128]
        wout_t = np.ascontiguousarray(
            wout3[:, hs, :].transpose(1, 2, 0).reshape(NH * VD, D)
        )  # [1024, 2048]
        per_g.append(
            {
                "wqd_t": wqd_t,
                "wqu_t": wqu_t,
                "wkvd_t": wkvd_t,
                "wupn": wupn,
                "wupv_t": wupv_t,
                "wout_t": wout_t,
                "cosf": cosf,
                "sinf": sinf,
            }
        )
    return per_g


def make_in_maps(inputs):
    x = np.asarray(inputs["x"], np.float32)
    per_g = _prep_shared(inputs)
    in_maps = []
    for c in range(N_CORES):
        b, g = c // 2, c % 2
        m = dict(per_g[g])
        m["x_t"] = np.ascontiguousarray(x[b].T)
        in_maps.append(m)
    return in_maps


def kernel(**inputs):
    nc = _get_nc()
    in_maps = make_in_maps(inputs)
    res = bass_utils.run_bass_kernel_spmd(
        nc, in_maps, core_ids=list(range(N_CORES))
    ).results
    out = np.empty((B, L, D), np.float32)
    for b in range(B):
        out[b] = (res[2 * b]["outT"] + res[2 * b + 1]["outT"]).T
    return out



# revision 9
# speedup vs baseline: 2.2607x; 2.2607x over previous
"""MLA prefill kernel for Trainium2, 8 NeuronCores.

Sharding: core c -> (batch b = c // 2, head-group g = c % 2). Each core
computes its batch's full sequence for its 8 heads, producing a partial
output (transposed, [2048, 1024]); the host sums the two head-group
partials per batch and transposes back.

Layout strategy (all on-chip matmuls contract over the partition dim):
  x arrives transposed ([D, L]) per batch, streamed in two d-halves;
  down projections produce latent-major activations.  The kv_up
  nope-slice is absorbed into the KV side: per head kv_abs[np, k] =
  Wabs_h^T kv_lat and V[k, vd] = kv_lat^T Wv_h are precomputed once, so
  each attention score tile is one 128-contraction matmul (+ a 64-wide
  rope matmul) and each value tile is a single matmul per key block.
  Attention runs "k-major" (scores^T [key, query]) with ideal causal
  packing: per key block only the valid query range is computed (F
  rounded up to >=256 to keep fp32r matmuls at full rate).  Softmax
  max-subtraction is skipped (scores are O(1)).  Diagonal blocks are
  masked post-exp with affine_select; denominators via ones-column
  matmuls; reciprocals via the fast DVE approximation.  The score path
  stays fp32r; the value path (V, probs, vn, Wout) is bf16 to fit SBUF
  (vn never round-trips through DRAM).
"""

import math
from contextlib import ExitStack

import ml_dtypes
import numpy as np

import concourse.bass as bass
import concourse.mybir as mybir
import concourse.tile as tile
from concourse import bacc, bass_utils

# ---- problem constants -------------------------------------------------
B, L, D = 4, 1024, 2048
H, NOPE, ROPE, VD, KVR = 16, 128, 64, 128, 512
DQ = 1024            # q latent dim
HD = NOPE + ROPE     # 192 per-head q dim
EPS = 1e-6
NH = 8               # heads per core
N_CORES = 8
SCALE = 1.0 / math.sqrt(NOPE + ROPE)

F32 = mybir.dt.float32
F32R = mybir.dt.float32r
BF16 = mybir.dt.bfloat16
DT = F32R

TOK = 512            # q-tile / PSUM bank width
NTOK = L // TOK      # 2
KB = 128             # key-token block
NKB = L // KB        # 8
ND = D // 128        # 16 contraction blocks over model dim
NLAT = DQ // 128     # 8 blocks over q latent
NKV = KVR // 128     # 4 blocks over kv latent
# down-proj output blocks: 8 q, 4 kv, 1 rope(64); kv+rope emitted first
OB_ORDER = [8, 9, 10, 11, 12] + list(range(8))


def _unit_table():
    """Per kb: list of (qb, q_start, F, diag) covering the valid causal
    query range, F rounded up to >=256 for full-rate fp32r matmuls."""
    units = {}
    for kb in range(NKB):
        k0 = kb * KB
        lst = []
        for qb in range(NTOK):
            q0 = qb * TOK
            if k0 >= q0 + TOK:
                continue  # fully masked
            if k0 + KB <= q0:
                lst.append((qb, q0, TOK, False))  # full block
            else:
                qs, f = k0, q0 + TOK - k0
                if f < 256:
                    qs, f = qs - (256 - f), 256
                lst.append((qb, qs, f, True))
        units[kb] = lst
    return units


UNITS = _unit_table()
FIRST_KB = {0: 0, 1: 0}
LAST_KB = {0: 3, 1: 7}


def build_nc():
    nc = bacc.Bacc("TRN2", target_bir_lowering=False, debug=False)

    t = {}
    t["x_t"] = nc.dram_tensor("x_t", [D, L], DT, kind="ExternalInput").ap()
    t["wd_t"] = nc.dram_tensor("wd_t", [D, 1664], DT, kind="ExternalInput").ap()
    t["wqu_t"] = nc.dram_tensor("wqu_t", [DQ, NH * HD], DT, kind="ExternalInput").ap()
    t["wabs_t"] = nc.dram_tensor(
        "wabs_t", [NH, 128, NKV, NOPE], DT, kind="ExternalInput"
    ).ap()
    t["wv_t"] = nc.dram_tensor(
        "wv_t", [128, NKV, NH * VD], DT, kind="ExternalInput"
    ).ap()
    t["wout_t"] = nc.dram_tensor(
        "wout_t", [NH * VD, D], BF16, kind="ExternalInput"
    ).ap()
    t["cosf"] = nc.dram_tensor("cosf", [128, L], F32, kind="ExternalInput").ap()
    t["sinf"] = nc.dram_tensor("sinf", [128, L], F32, kind="ExternalInput").ap()
    t["outT"] = nc.dram_tensor("outT", [D, L], F32, kind="ExternalOutput").ap()

    with tile.TileContext(nc) as tc:
        _emit(tc, t)
    nc.compile()
    return nc


def _evict(nc, i, dst, src):
    """PSUM -> SBUF eviction, alternating engines to balance load."""
    if i % 2 == 0:
        nc.scalar.copy(dst, src)
    else:
        nc.vector.tensor_copy(dst, src)


def _rms(tc, ps_d, prow, psq, ones_col_r, eps_t, lat, nlb, dim):
    """RMS-normalize `lat` ([128, nlb, L], latent-major, fp32r) in place."""
    nc = tc.nc
    for tk in range(NTOK):
        ts = slice(tk * TOK, (tk + 1) * TOK)
        ps_ssq = ps_d.tile([1, TOK], F32, tag="d")
        for lb in range(nlb):
            sq = psq.tile([128, TOK], DT, tag="sq")
            sl = lat[:, lb, ts]
            nc.vector.tensor_mul(sq, sl, sl)
            nc.tensor.matmul(
                ps_ssq, ones_col_r, sq, start=(lb == 0), stop=(lb == nlb - 1)
            )
        rt = prow.tile([1, TOK], F32, tag="rt")
        nc.scalar.activation(
            rt, ps_ssq, mybir.ActivationFunctionType.Sqrt,
            bias=eps_t, scale=1.0 / dim,
        )
        rr = prow.tile([1, TOK], F32, tag="rr")
        nc.vector.reciprocal_approx_fast(out=rr, in_=rt)
        rb = prow.tile([128, TOK], F32, tag="rb")
        nc.gpsimd.partition_broadcast(rb, rr)
        for lb in range(nlb):
            sl = lat[:, lb, ts]
            nc.vector.tensor_mul(sl, sl, rb)


def _emit(tc, t):
    nc = tc.nc
    with ExitStack() as c0:
        c0.enter_context(
            nc.allow_low_precision(reason="fp32r/bf16 rounding is intentional")
        )
        from concourse import library_config

        nc.gpsimd.load_library(library_config.attnmlp)

        glob = c0.enter_context(tc.tile_pool(name="glob", bufs=1))
        ps_mm = c0.enter_context(tc.tile_pool(name="ps_mm", bufs=4, space="PSUM"))
        ps_v = c0.enter_context(tc.tile_pool(name="ps_v", bufs=2, space="PSUM"))
        ps_d = c0.enter_context(tc.tile_pool(name="ps_d", bufs=2, space="PSUM"))

        # ---- constants -----------------------------------------------
        ones_f32 = glob.tile([128, 128], F32, tag="ones32")
        nc.vector.memset(ones_f32, 1.0)
        ones_col_r = glob.tile([128, 1], DT, tag="onesr_c")
        nc.vector.tensor_copy(ones_col_r, ones_f32[:, :1])
        ones_col_b = glob.tile([128, 1], BF16, tag="onesb_c")
        nc.vector.tensor_copy(ones_col_b, ones_f32[:, :1])
        eps_t = glob.tile([1, 1], F32, tag="eps")
        nc.vector.memset(eps_t, EPS)
        k_roped = glob.tile([128, L], BF16, tag="kroped")
        v_all = glob.tile([128, NKB, NH * VD], BF16, tag="vall")

        with ExitStack() as cL:
            pL = cL.enter_context(tc.tile_pool(name="pL", bufs=1))
            cosf = pL.tile([128, L], F32, tag="cosf")
            nc.sync.dma_start(out=cosf, in_=t["cosf"])
            sinf = pL.tile([128, L], F32, tag="sinf")
            nc.sync.dma_start(out=sinf, in_=t["sinf"])
            kv_lat = pL.tile([128, NKV, L], DT, tag="kvlat")
            q_lat = pL.tile([128, NLAT, L], DT, tag="qlat")

            # ---- phase X: down projections (x in two d-halves) -------
            with ExitStack() as cX:
                px = cX.enter_context(tc.tile_pool(name="px", bufs=2))
                pwd = cX.enter_context(tc.tile_pool(name="pwd", bufs=3))
                prow = cX.enter_context(tc.tile_pool(name="prow", bufs=1))
                psq = cX.enter_context(tc.tile_pool(name="psq", bufs=3))
                pkr = cX.enter_context(tc.tile_pool(name="pkr", bufs=1))

                x_r = t["x_t"].rearrange("(b p) t -> p b t", p=128)
                wd_r = t["wd_t"].rearrange("(b p) m -> p b m", p=128)
                kr_pair = pkr.tile([128, 2, L], BF16, tag="krpair")

                warm_f = pkr.tile([128, TOK], F32, tag="warmf")
                nc.vector.memset(warm_f, 0.0)
                warm = pkr.tile([128, TOK], DT, tag="warm")
                nc.vector.tensor_copy(warm, warm_f)
                for _ in range(20):
                    ps = ps_mm.tile([128, TOK], F32, tag="mm")
                    nc.tensor.matmul(ps, warm[:, :128], warm)

                for half in range(2):
                    hs = slice(half * 8, half * 8 + 8)
                    xh = px.tile([128, 8, L], DT, tag="xh")
                    for ch in range(4):
                        nc.sync.dma_start(
                            out=xh[:, ch * 2 : ch * 2 + 2, :],
                            in_=x_r[:, half * 8 + ch * 2 : half * 8 + ch * 2 + 2, :],
                        )
                    for ob in OB_ORDER:
                        cw = 64 if ob == 12 else 128
                        wd = pwd.tile([128, 8, 128], DT, tag="wd")
                        nc.sync.dma_start(
                            out=wd[:, :, :cw],
                            in_=wd_r[:, hs, ob * 128 : ob * 128 + cw],
                        )
                        pss = [
                            ps_mm.tile([128, TOK], F32, tag="mm",
                                       name=f"psd{tk}")
                            for tk in range(NTOK)
                        ]
                        for db in range(8):
                            for tk in range(NTOK):
                                ts = slice(tk * TOK, (tk + 1) * TOK)
                                nc.tensor.matmul(
                                    pss[tk][:cw], wd[:, db, :cw], xh[:, db, ts],
                                    start=(db == 0), stop=(db == 7),
                                )
                        for tk in range(NTOK):
                            ts = slice(tk * TOK, (tk + 1) * TOK)
                            if ob < 8:
                                dst = q_lat[:, ob, ts]
                            elif ob < 12:
                                dst = kv_lat[:, ob - 8, ts]
                            else:
                                dst = kr_pair[:64, 0, ts]
                            if half == 0:
                                _evict(nc, ob + tk, dst, pss[tk][:cw])
                            else:
                                nc.vector.tensor_add(dst, dst, pss[tk][:cw])

                        if half == 1 and ob == 12:
                            # kv RMS + k rope (overlaps 2nd-half q blocks)
                            _rms(tc, ps_d, prow, psq, ones_col_r,
                                 eps_t, kv_lat, NKV, KVR)
                            nc.sync.dma_start(
                                out=kr_pair[0:32, 1, :], in_=kr_pair[32:64, 0, :]
                            )
                            nc.sync.dma_start(
                                out=kr_pair[32:64, 1, :], in_=kr_pair[0:32, 0, :]
                            )
                            nc.vector.tensor_mul(
                                k_roped[0:64], kr_pair[0:64, 0, :], cosf[0:64]
                            )
                            nc.vector.tensor_mul(
                                kr_pair[0:64, 0, :], kr_pair[0:64, 1, :],
                                sinf[0:64],
                            )
                            nc.vector.tensor_add(
                                k_roped[0:64], k_roped[0:64], kr_pair[0:64, 0, :]
                            )
                            nc.sync.dma_start(
                                out=k_roped[64:128], in_=k_roped[0:64]
                            )

                # q RMS (resolves while kv_abs/V matmuls run)
                _rms(tc, ps_d, prow, psq, ones_col_r,
                     eps_t, q_lat, NLAT, DQ)

            # ---- phases P/Q/A under attention-lived pools ------------
            pwo = cL.enter_context(tc.tile_pool(name="pwo", bufs=3))
            pvn = cL.enter_context(tc.tile_pool(name="pvn", bufs=1))
            vn = pvn.tile([128, NH, L], BF16, tag="vn")
            wout_r = t["wout_t"].rearrange("(b p) m -> p b m", p=128)
            wouts = [None] * 16

            def fetch_wout(c):
                wouts[c] = pwo.tile([128, NH, 128], BF16, tag="wout", name=f"wout{c}")
                nc.sync.dma_start(
                    out=wouts[c], in_=wout_r[:, :, c * 128 : (c + 1) * 128]
                )

            with ExitStack() as cM:
                pM = cM.enter_context(tc.tile_pool(name="pM", bufs=1))
                kv_abs = pM.tile([128, NH, L], DT, tag="kvabs")
                qT_nope = pM.tile([128, NH, L], DT, tag="qnope")
                q_roped = pM.tile([128, NH // 2, L], BF16, tag="qroped")

                # ---- phase P: kv_abs + V precompute ------------------
                with ExitStack() as cP:
                    pw = cP.enter_context(tc.tile_pool(name="pw", bufs=2))
                    for h in range(NH):
                        wabs = pw.tile([128, NKV, NOPE], DT, tag="wabs")
                        nc.sync.dma_start(out=wabs, in_=t["wabs_t"][h])
                        pss = [
                            ps_mm.tile([128, TOK], F32, tag="mm",
                                       name=f"psp{tk}")
                            for tk in range(NTOK)
                        ]
                        for lb in range(NKV):
                            for tk in range(NTOK):
                                ts = slice(tk * TOK, (tk + 1) * TOK)
                                nc.tensor.matmul(
                                    pss[tk], wabs[:, lb], kv_lat[:, lb, ts],
                                    start=(lb == 0), stop=(lb == NKV - 1),
                                )
                        for tk in range(NTOK):
                            ts = slice(tk * TOK, (tk + 1) * TOK)
                            _evict(nc, h + tk, kv_abs[:, h, ts], pss[tk])

                    for qc in range(4):
                        hv = slice(qc * 256, (qc + 1) * 256)
                        wv = pw.tile([128, NKV, 256], DT, tag="wv")
                        nc.sync.dma_start(out=wv, in_=t["wv_t"][:, :, hv])
                        for kp in range(NKB // 2):
                            pss = [
                                ps_mm.tile([128, 256], F32, tag="mm",
                                           name=f"psv{ki}")
                                for ki in range(2)
                            ]
                            for lb in range(NKV):
                                for ki in range(2):
                                    kb = kp * 2 + ki
                                    ks = slice(kb * KB, (kb + 1) * KB)
                                    nc.tensor.matmul(
                                        pss[ki], kv_lat[:, lb, ks],
                                        wv[:, lb, :],
                                        start=(lb == 0), stop=(lb == NKV - 1),
                                    )
                            for ki in range(2):
                                kb = kp * 2 + ki
                                _evict(nc, qc + kb, v_all[:, kb, hv], pss[ki])

                # ---- phase Q: q up-projection + q rope ---------------
                with ExitStack() as cQ:
                    pqu = cQ.enter_context(tc.tile_pool(name="pqu", bufs=2))
                    ppair = cQ.enter_context(tc.tile_pool(name="ppair", bufs=1))
                    wqu_r = t["wqu_t"].rearrange("(b p) m -> p b m", p=128)
                    for p in range(NH // 2):
                        q_pair = ppair.tile([128, 2, L], BF16, tag="pair")
                        for piece in range(3):
                            col0 = p * 384 + piece * 128
                            wqu = pqu.tile([128, NLAT, 128], DT, tag="wqu")
                            nc.sync.dma_start(
                                out=wqu, in_=wqu_r[:, :, col0 : col0 + 128]
                            )
                            pss = [
                                ps_mm.tile([128, TOK], F32, tag="mm",
                                           name=f"psq{tk}")
                                for tk in range(NTOK)
                            ]
                            for lb in range(NLAT):
                                for tk in range(NTOK):
                                    ts = slice(tk * TOK, (tk + 1) * TOK)
                                    nc.tensor.matmul(
                                        pss[tk], wqu[:, lb], q_lat[:, lb, ts],
                                        start=(lb == 0), stop=(lb == NLAT - 1),
                                    )
                            for tk in range(NTOK):
                                ts = slice(tk * TOK, (tk + 1) * TOK)
                                if piece < 2:
                                    _evict(nc, p + piece + tk,
                                           qT_nope[:, 2 * p + piece, ts],
                                           pss[tk])
                                else:
                                    _evict(nc, p + tk, q_pair[:, 0, ts],
                                           pss[tk])
                        nc.sync.dma_start(
                            out=q_pair[0:32, 1, :], in_=q_pair[32:64, 0, :]
                        )
                        nc.sync.dma_start(
                            out=q_pair[32:64, 1, :], in_=q_pair[0:32, 0, :]
                        )
                        nc.sync.dma_start(
                            out=q_pair[64:96, 1, :], in_=q_pair[96:128, 0, :]
                        )
                        nc.sync.dma_start(
                            out=q_pair[96:128, 1, :], in_=q_pair[64:96, 0, :]
                        )
                        nc.vector.tensor_mul(
                            q_roped[:, p, :], q_pair[:, 0, :], cosf
                        )
                        nc.vector.tensor_mul(
                            q_pair[:, 0, :], q_pair[:, 1, :], sinf
                        )
                        nc.vector.tensor_add(
                            q_roped[:, p, :], q_roped[:, p, :], q_pair[:, 0, :]
                        )

                # ---- phase A: attention ------------------------------
                with ExitStack() as cA:
                    pe = cA.enter_context(tc.tile_pool(name="pe", bufs=3))
                    pvr = cA.enter_context(tc.tile_pool(name="pvr", bufs=2))
                    prd = cA.enter_context(tc.tile_pool(name="prd", bufs=2))

                    for h in range(NH):
                        hb = (h % 2) * 64
                        pr = h // 2
                        hv = slice(h * VD, (h + 1) * VD)
                        ps_vt = {}
                        ps_dt = {}
                        for qb in range(NTOK):
                            ps_vt[qb] = ps_v.tile([128, TOK], F32, tag="v", name=f"psvt{qb}")
                            ps_dt[qb] = ps_d.tile([1, TOK], F32, tag="d", name=f"psdt{qb}")
                        for kb in range(NKB):
                            k0 = kb * KB
                            ks = slice(k0, k0 + KB)
                            us = UNITS[kb]
                            sts = [
                                ps_mm.tile([128, TOK], F32, tag="mm",
                                           name=f"pss{ui}")
                                for ui in range(len(us))
                            ]
                            for (qb, qs, f, dg), st in zip(us, sts):
                                nc.tensor.matmul(
                                    st[:, :f], kv_abs[:, h, ks],
                                    qT_nope[:, h, qs : qs + f],
                                    start=True, stop=False,
                                )
                            for (qb, qs, f, dg), st in zip(us, sts):
                                nc.tensor.matmul(
                                    st[:, :f], k_roped[hb : hb + 64, ks],
                                    q_roped[hb : hb + 64, pr, qs : qs + f],
                                    start=False, stop=True,
                                )
                            ets = []
                            for (qb, qs, f, dg), st in zip(us, sts):
                                e_t = pe.tile([128, TOK], BF16, tag="e")
                                nc.scalar.activation(
                                    e_t[:, :f], st[:, :f],
                                    mybir.ActivationFunctionType.Exp,
                                    scale=SCALE,
                                )
                                if dg:
                                    nc.gpsimd.affine_select(
                                        out=e_t[:, :f], in_=e_t[:, :f],
                                        pattern=[[1, f]],
                                        compare_op=mybir.AluOpType.is_ge,
                                        fill=0.0,
                                        base=qs - k0,
                                        channel_multiplier=-1,
                                    )
                                ets.append(e_t)
                            for (qb, qs, f, dg), e_t in zip(us, ets):
                                lo = qs - qb * TOK
                                nc.tensor.matmul(
                                    ps_dt[qb][:, lo : lo + f], ones_col_b,
                                    e_t[:, :f],
                                    start=(kb == FIRST_KB[qb]),
                                    stop=(kb == LAST_KB[qb]),
                                )
                            for (qb, qs, f, dg), e_t in zip(us, ets):
                                lo = qs - qb * TOK
                                nc.tensor.matmul(
                                    ps_vt[qb][:, lo : lo + f],
                                    v_all[:, kb, hv], e_t[:, :f],
                                    start=(kb == FIRST_KB[qb]),
                                    stop=(kb == LAST_KB[qb]),
                                )
                            for qb in range(NTOK):
                                if kb == LAST_KB[qb]:
                                    ts = slice(qb * TOK, (qb + 1) * TOK)
                                    rd = prd.tile([1, TOK], F32, tag="rd")
                                    nc.vector.reciprocal_approx_fast(
                                        out=rd, in_=ps_dt[qb]
                                    )
                                    rb = prd.tile([128, TOK], F32, tag="rb")
                                    nc.gpsimd.partition_broadcast(rb, rd)
                                    vraw = pvr.tile([128, TOK], F32, tag="vr")
                                    nc.scalar.copy(vraw, ps_vt[qb])
                                    nc.vector.tensor_mul(
                                        vn[:, h, ts], vraw, rb
                                    )
                        if h == 6:
                            fetch_wout(0)
                            fetch_wout(1)

            # ---- phase O: output projection (cM closed) --------------
            with ExitStack() as cO:
                po = cO.enter_context(tc.tile_pool(name="po", bufs=3))
                for c in range(16):
                    if wouts[c] is None:
                        fetch_wout(c)
                    row = c * 128
                    pss = [
                        ps_mm.tile([128, TOK], F32, tag="mm",
                                   name=f"pso{tk}")
                        for tk in range(NTOK)
                    ]
                    for hbk in range(NH):
                        for tk in range(NTOK):
                            ts = slice(tk * TOK, (tk + 1) * TOK)
                            nc.tensor.matmul(
                                pss[tk], wouts[c][:, hbk, :], vn[:, hbk, ts],
                                start=(hbk == 0), stop=(hbk == NH - 1),
                            )
                    for tk in range(NTOK):
                        ts = slice(tk * TOK, (tk + 1) * TOK)
                        o_t = po.tile([128, TOK], F32, tag="o")
                        _evict(nc, c + tk, o_t, pss[tk])
                        nc.sync.dma_start(
                            out=t["outT"][row : row + 128, ts], in_=o_t
                        )


# ======================================================================
# host side
# ======================================================================

_NC_CACHE = {}


def _get_nc():
    if "nc" not in _NC_CACHE:
        _NC_CACHE["nc"] = build_nc()
    return _NC_CACHE["nc"]


def _prep_shared(inputs):
    wq_down = np.asarray(inputs["Wq_down"], np.float32)
    wq_up = np.asarray(inputs["Wq_up"], np.float32)
    wkv_down = np.asarray(inputs["Wkv_down"], np.float32)
    wkv_up = np.asarray(inputs["Wkv_up"], np.float32)
    wout = np.asarray(inputs["Wout"], np.float32)
    rms_q_w = np.asarray(inputs["rms_q_w"], np.float32)
    rms_kv_w = np.asarray(inputs["rms_kv_w"], np.float32)
    freq = np.asarray(inputs["freq_cis"], np.float32)  # [L, 32, 2]

    # split re/im layout for all rope dims: re parts first, then im parts
    rope_perm = np.concatenate(
        [np.arange(0, ROPE, 2), np.arange(1, ROPE, 2)]
    )  # [64]

    # combined down-proj: q latent | kv latent | k-rope (re/im split), pad
    wd = np.zeros((1664, D), np.float32)
    wd[:DQ] = wq_down
    wd[DQ : DQ + KVR] = wkv_down[:KVR]
    wd[DQ + KVR : DQ + KVR + ROPE] = wkv_down[KVR:][rope_perm]
    wd_t = np.ascontiguousarray(wd.T)  # [D, 1664]

    # rope tables (dim-major, split re/im, duplicated partition halves)
    cos = freq[:, :, 0].T  # [32, L]
    sin = freq[:, :, 1].T
    cosf64 = np.vstack([cos, cos])  # [64, L]
    sinf64 = np.vstack([-sin, sin])
    cosf = np.ascontiguousarray(np.vstack([cosf64, cosf64]))  # [128, L]
    sinf = np.ascontiguousarray(np.vstack([sinf64, sinf64]))

    wq_up3 = (wq_up * rms_q_w[None, :]).reshape(H, HD, DQ)
    wq_nope = wq_up3[:, :NOPE, :]                      # [H, 128, DQ]
    wq_rope = wq_up3[:, NOPE:, :][:, rope_perm, :]     # [H, 64, DQ]
    wkv_up3 = wkv_up.reshape(H, NOPE + VD, KVR)
    wout3 = wout.reshape(D, H, VD)

    per_g = []
    for g in range(2):
        hs = list(range(g * NH, (g + 1) * NH))
        # q up: per pair [nope(2p) | nope(2p+1) | rope(2p)+rope(2p+1)]
        cols = []
        for p in range(NH // 2):
            h0, h1 = hs[2 * p], hs[2 * p + 1]
            cols.append(wq_nope[h0])
            cols.append(wq_nope[h1])
            cols.append(wq_rope[h0])
            cols.append(wq_rope[h1])
        wqu_t = np.ascontiguousarray(
            np.concatenate(cols, axis=0).T
        )  # [DQ, 1536]

        wabs = wkv_up3[hs, :NOPE, :] * rms_kv_w[None, None, :]  # [8,128,512]
        # per head: [KVR, NOPE] -> [128, 4, 128]
        wabs_t = np.ascontiguousarray(
            wabs.transpose(0, 2, 1).reshape(NH, NKV, 128, NOPE)
            .transpose(0, 2, 1, 3)
        )  # [8, 128, 4, 128]

        wv = wkv_up3[hs, NOPE:, :] * rms_kv_w[None, None, :]  # [8, 128, 512]
        # [KVR, NH*VD] -> [128, 4, 1024]
        wv_t = np.ascontiguousarray(
            wv.transpose(2, 0, 1).reshape(NKV, 128, NH * VD)
            .transpose(1, 0, 2)
        )  # [128, 4, 1024]

        wout_t = np.ascontiguousarray(
            wout3[:, hs, :].transpose(1, 2, 0).reshape(NH * VD, D)
        ).astype(ml_dtypes.bfloat16)  # [1024, 2048]
        per_g.append(
            {
                "wd_t": wd_t,
                "wqu_t": wqu_t,
                "wabs_t": wabs_t,
                "wv_t": wv_t,
                "wout_t": wout_t,
                "cosf": cosf,
                "sinf": sinf,
            }
        )
    return per_g


def make_in_maps(inputs):
    x = np.asarray(inputs["x"], np.float32)
    per_g = _prep_shared(inputs)
    in_maps = []
    for c in range(N_CORES):
        b, g = c // 2, c % 2
        m = dict(per_g[g])
        m["x_t"] = np.ascontiguousarray(x[b].T)
        in_maps.append(m)
    return in_maps


def kernel(**inputs):
    nc = _get_nc()
    in_maps = make_in_maps(inputs)
    res = bass_utils.run_bass_kernel_spmd(
        nc, in_maps, core_ids=list(range(N_CORES))
    ).results
    out = np.empty((B, L, D), np.float32)
    for b in range(B):
        out[b] = (res[2 * b]["outT"] + res[2 * b + 1]["outT"]).T
    return out


# revision 11
# speedup vs baseline: 2.3079x; 1.0209x over previous
"""MLA prefill kernel for Trainium2, 8 NeuronCores.

Sharding: core c -> (batch b = c // 2, head-group g = c % 2). Each core
computes its batch's full sequence for its 8 heads, producing a partial
output (transposed, [2048, 1024]); the host sums the two head-group
partials per batch and transposes back.

Layout strategy (all on-chip matmuls contract over the partition dim):
  x arrives transposed ([D, L]) per batch, streamed in two d-halves;
  down projections produce latent-major activations.  The kv_up
  nope-slice is absorbed into the KV side: per head kv_abs[np, k] =
  Wabs_h^T kv_lat and V[k, vd] = kv_lat^T Wv_h are precomputed once, so
  each attention score tile is one 128-contraction matmul (+ a 64-wide
  rope matmul) and each value tile is a single matmul per key block.
  Attention runs "k-major" (scores^T [key, query]) with ideal causal
  packing: per key block only the valid query range is computed (F
  rounded up to >=256 to keep fp32r matmuls at full rate).  Softmax
  max-subtraction is skipped (scores are O(1)).  Diagonal blocks are
  masked post-exp with affine_select; denominators via ones-column
  matmuls; reciprocals via the fast DVE approximation.  The score path
  stays fp32r; the value path (V, probs, vn, Wout) is bf16 to fit SBUF
  (vn never round-trips through DRAM).
"""

import math
from contextlib import ExitStack

import ml_dtypes
import numpy as np

import concourse.bass as bass
import concourse.mybir as mybir
import concourse.tile as tile
from concourse import bacc, bass_utils

# ---- problem constants -------------------------------------------------
B, L, D = 4, 1024, 2048
H, NOPE, ROPE, VD, KVR = 16, 128, 64, 128, 512
DQ = 1024            # q latent dim
HD = NOPE + ROPE     # 192 per-head q dim
EPS = 1e-6
NH = 8               # heads per core
N_CORES = 8
SCALE = 1.0 / math.sqrt(NOPE + ROPE)

F32 = mybir.dt.float32
F32R = mybir.dt.float32r
BF16 = mybir.dt.bfloat16
DT = F32R

TOK = 512            # q-tile / PSUM bank width
NTOK = L // TOK      # 2
KB = 128             # key-token block
NKB = L // KB        # 8
ND = D // 128        # 16 contraction blocks over model dim
NLAT = DQ // 128     # 8 blocks over q latent
NKV = KVR // 128     # 4 blocks over kv latent
# down-proj output blocks: 8 q, 4 kv, 1 rope(64); kv+rope emitted first
OB_ORDER = [8, 9, 10, 11, 12] + list(range(8))


def _unit_table():
    """Per kb: list of (qb, q_start, F, diag) covering the valid causal
    query range, F rounded up to >=256 for full-rate fp32r matmuls."""
    units = {}
    for kb in range(NKB):
        k0 = kb * KB
        lst = []
        for qb in range(NTOK):
            q0 = qb * TOK
            if k0 >= q0 + TOK:
                continue  # fully masked
            if k0 + KB <= q0:
                lst.append((qb, q0, TOK, False))  # full block
            else:
                qs, f = k0, q0 + TOK - k0
                if f < 256:
                    qs, f = qs - (256 - f), 256
                lst.append((qb, qs, f, True))
        units[kb] = lst
    return units


UNITS = _unit_table()
FIRST_KB = {0: 0, 1: 0}
LAST_KB = {0: 3, 1: 7}


def build_nc():
    nc = bacc.Bacc("TRN2", target_bir_lowering=False, debug=False)

    t = {}
    t["x_t"] = nc.dram_tensor("x_t", [D, L], BF16, kind="ExternalInput").ap()
    t["wd_t"] = nc.dram_tensor("wd_t", [D, 1664], BF16, kind="ExternalInput").ap()
    t["wqu_t"] = nc.dram_tensor("wqu_t", [DQ, NH * HD], DT, kind="ExternalInput").ap()
    t["wabs_t"] = nc.dram_tensor(
        "wabs_t", [NH, 128, NKV, NOPE], DT, kind="ExternalInput"
    ).ap()
    t["wv_t"] = nc.dram_tensor(
        "wv_t", [128, NKV, NH * VD], DT, kind="ExternalInput"
    ).ap()
    t["wout_t"] = nc.dram_tensor(
        "wout_t", [NH * VD, D], BF16, kind="ExternalInput"
    ).ap()
    t["cosf"] = nc.dram_tensor("cosf", [128, L], F32, kind="ExternalInput").ap()
    t["sinf"] = nc.dram_tensor("sinf", [128, L], F32, kind="ExternalInput").ap()
    t["outT"] = nc.dram_tensor("outT", [D, L], F32, kind="ExternalOutput").ap()

    with tile.TileContext(nc) as tc:
        _emit(tc, t)
    nc.compile()
    return nc


def _evict(nc, i, dst, src):
    """PSUM -> SBUF eviction, alternating engines to balance load."""
    if i % 2 == 0:
        nc.scalar.copy(dst, src)
    else:
        nc.vector.tensor_copy(dst, src)


def _rms(tc, ps_d, prow, psq, ones_col_r, eps_t, lat, nlb, dim):
    """RMS-normalize `lat` ([128, nlb, L], latent-major, fp32r) in place."""
    nc = tc.nc
    for tk in range(NTOK):
        ts = slice(tk * TOK, (tk + 1) * TOK)
        ps_ssq = ps_d.tile([1, TOK], F32, tag="d")
        for lb in range(nlb):
            sq = psq.tile([128, TOK], DT, tag="sq")
            sl = lat[:, lb, ts]
            nc.vector.tensor_mul(sq, sl, sl)
            nc.tensor.matmul(
                ps_ssq, ones_col_r, sq, start=(lb == 0), stop=(lb == nlb - 1)
            )
        rt = prow.tile([1, TOK], F32, tag="rt")
        nc.scalar.activation(
            rt, ps_ssq, mybir.ActivationFunctionType.Sqrt,
            bias=eps_t, scale=1.0 / dim,
        )
        rr = prow.tile([1, TOK], F32, tag="rr")
        nc.vector.reciprocal_approx_fast(out=rr, in_=rt)
        rb = prow.tile([128, TOK], F32, tag="rb")
        nc.gpsimd.partition_broadcast(rb, rr)
        for lb in range(nlb):
            sl = lat[:, lb, ts]
            nc.vector.tensor_mul(sl, sl, rb)


def _emit(tc, t):
    nc = tc.nc
    with ExitStack() as c0:
        c0.enter_context(
            nc.allow_low_precision(reason="fp32r/bf16 rounding is intentional")
        )
        from concourse import library_config

        nc.gpsimd.load_library(library_config.attnmlp)

        glob = c0.enter_context(tc.tile_pool(name="glob", bufs=1))
        ps_mm = c0.enter_context(tc.tile_pool(name="ps_mm", bufs=4, space="PSUM"))
        ps_v = c0.enter_context(tc.tile_pool(name="ps_v", bufs=2, space="PSUM"))
        ps_d = c0.enter_context(tc.tile_pool(name="ps_d", bufs=2, space="PSUM"))

        # ---- constants -----------------------------------------------
        ones_f32 = glob.tile([128, 128], F32, tag="ones32")
        nc.vector.memset(ones_f32, 1.0)
        ones_col_r = glob.tile([128, 1], DT, tag="onesr_c")
        nc.vector.tensor_copy(ones_col_r, ones_f32[:, :1])
        ones_col_b = glob.tile([128, 1], BF16, tag="onesb_c")
        nc.vector.tensor_copy(ones_col_b, ones_f32[:, :1])
        eps_t = glob.tile([1, 1], F32, tag="eps")
        nc.vector.memset(eps_t, EPS)
        k_roped = glob.tile([128, L], BF16, tag="kroped")
        v_all = glob.tile([128, NKB, NH * VD], BF16, tag="vall")

        with ExitStack() as cL:
            pL = cL.enter_context(tc.tile_pool(name="pL", bufs=1))
            cosf = pL.tile([128, L], F32, tag="cosf")
            nc.sync.dma_start(out=cosf, in_=t["cosf"])
            sinf = pL.tile([128, L], F32, tag="sinf")
            nc.sync.dma_start(out=sinf, in_=t["sinf"])
            kv_lat = pL.tile([128, NKV, L], DT, tag="kvlat")
            q_lat = pL.tile([128, NLAT, L], DT, tag="qlat")

            # ---- phase X: down projections (x in two d-halves) -------
            with ExitStack() as cX:
                px = cX.enter_context(tc.tile_pool(name="px", bufs=1))
                pwd = cX.enter_context(tc.tile_pool(name="pwd", bufs=3))
                prow = cX.enter_context(tc.tile_pool(name="prow", bufs=1))
                psq = cX.enter_context(tc.tile_pool(name="psq", bufs=3))
                pkr = cX.enter_context(tc.tile_pool(name="pkr", bufs=1))

                x_r = t["x_t"].rearrange("(b p) t -> p b t", p=128)
                wd_r = t["wd_t"].rearrange("(b p) m -> p b m", p=128)
                kr_pair = pkr.tile([128, 2, L], BF16, tag="krpair")

                warm_f = pkr.tile([128, TOK], F32, tag="warmf")
                nc.vector.memset(warm_f, 0.0)
                warm = pkr.tile([128, TOK], DT, tag="warm")
                nc.vector.tensor_copy(warm, warm_f)
                for _ in range(20):
                    ps = ps_mm.tile([128, TOK], F32, tag="mm")
                    nc.tensor.matmul(ps, warm[:, :128], warm)

                x_sb = px.tile([128, ND, L], BF16, tag="x")
                for ch in range(8):
                    nc.sync.dma_start(
                        out=x_sb[:, ch * 2 : ch * 2 + 2, :],
                        in_=x_r[:, ch * 2 : ch * 2 + 2, :],
                    )
                for ob in OB_ORDER:
                    cw = 64 if ob == 12 else 128
                    wd = pwd.tile([128, ND, 128], BF16, tag="wd")
                    nc.sync.dma_start(
                        out=wd[:, :, :cw],
                        in_=wd_r[:, :, ob * 128 : ob * 128 + cw],
                    )
                    pss = [
                        ps_mm.tile([128, TOK], F32, tag="mm",
                                   name=f"psd{tk}")
                        for tk in range(NTOK)
                    ]
                    for db in range(ND):
                        for tk in range(NTOK):
                            ts = slice(tk * TOK, (tk + 1) * TOK)
                            nc.tensor.matmul(
                                pss[tk][:cw], wd[:, db, :cw], x_sb[:, db, ts],
                                start=(db == 0), stop=(db == ND - 1),
                            )
                    for tk in range(NTOK):
                        ts = slice(tk * TOK, (tk + 1) * TOK)
                        if ob < 8:
                            dst = q_lat[:, ob, ts]
                        elif ob < 12:
                            dst = kv_lat[:, ob - 8, ts]
                        else:
                            dst = kr_pair[:64, 0, ts]
                        _evict(nc, ob + tk, dst, pss[tk][:cw])

                    if ob == 12:
                        # kv RMS + k rope (overlaps q blocks)
                        _rms(tc, ps_d, prow, psq, ones_col_r,
                             eps_t, kv_lat, NKV, KVR)
                        nc.sync.dma_start(
                            out=kr_pair[0:32, 1, :], in_=kr_pair[32:64, 0, :]
                        )
                        nc.sync.dma_start(
                            out=kr_pair[32:64, 1, :], in_=kr_pair[0:32, 0, :]
                        )
                        nc.vector.tensor_mul(
                            k_roped[0:64], kr_pair[0:64, 0, :], cosf[0:64]
                        )
                        nc.vector.tensor_mul(
                            kr_pair[0:64, 0, :], kr_pair[0:64, 1, :],
                            sinf[0:64],
                        )
                        nc.vector.tensor_add(
                            k_roped[0:64], k_roped[0:64], kr_pair[0:64, 0, :]
                        )
                        nc.sync.dma_start(
                            out=k_roped[64:128], in_=k_roped[0:64]
                        )

                # q RMS (resolves while kv_abs/V matmuls run)
                _rms(tc, ps_d, prow, psq, ones_col_r,
                     eps_t, q_lat, NLAT, DQ)

            # ---- phases P/Q/A under attention-lived pools ------------
            pwo = cL.enter_context(tc.tile_pool(name="pwo", bufs=3))
            pvn = cL.enter_context(tc.tile_pool(name="pvn", bufs=1))
            vn = pvn.tile([128, NH, L], BF16, tag="vn")
            wout_r = t["wout_t"].rearrange("(b p) m -> p b m", p=128)
            wouts = [None] * 16

            def fetch_wout(c):
                wouts[c] = pwo.tile([128, NH, 128], BF16, tag="wout", name=f"wout{c}")
                nc.sync.dma_start(
                    out=wouts[c], in_=wout_r[:, :, c * 128 : (c + 1) * 128]
                )

            with ExitStack() as cM:
                pM = cM.enter_context(tc.tile_pool(name="pM", bufs=1))
                kv_abs = pM.tile([128, NH, L], DT, tag="kvabs")
                qT_nope = pM.tile([128, NH, L], DT, tag="qnope")
                q_roped = pM.tile([128, NH // 2, L], BF16, tag="qroped")

                # ---- phase P: kv_abs + V precompute ------------------
                with ExitStack() as cP:
                    pw = cP.enter_context(tc.tile_pool(name="pw", bufs=2))
                    for h in range(NH):
                        wabs = pw.tile([128, NKV, NOPE], DT, tag="wabs")
                        nc.sync.dma_start(out=wabs, in_=t["wabs_t"][h])
                        pss = [
                            ps_mm.tile([128, TOK], F32, tag="mm",
                                       name=f"psp{tk}")
                            for tk in range(NTOK)
                        ]
                        for lb in range(NKV):
                            for tk in range(NTOK):
                                ts = slice(tk * TOK, (tk + 1) * TOK)
                                nc.tensor.matmul(
                                    pss[tk], wabs[:, lb], kv_lat[:, lb, ts],
                                    start=(lb == 0), stop=(lb == NKV - 1),
                                )
                        for tk in range(NTOK):
                            ts = slice(tk * TOK, (tk + 1) * TOK)
                            _evict(nc, h + tk, kv_abs[:, h, ts], pss[tk])

                    for qc in range(4):
                        hv = slice(qc * 256, (qc + 1) * 256)
                        wv = pw.tile([128, NKV, 256], DT, tag="wv")
                        nc.sync.dma_start(out=wv, in_=t["wv_t"][:, :, hv])
                        for kp in range(NKB // 2):
                            pss = [
                                ps_mm.tile([128, 256], F32, tag="mm",
                                           name=f"psv{ki}")
                                for ki in range(2)
                            ]
                            for lb in range(NKV):
                                for ki in range(2):
                                    kb = kp * 2 + ki
                                    ks = slice(kb * KB, (kb + 1) * KB)
                                    nc.tensor.matmul(
                                        pss[ki], kv_lat[:, lb, ks],
                                        wv[:, lb, :],
                                        start=(lb == 0), stop=(lb == NKV - 1),
                                    )
                            for ki in range(2):
                                kb = kp * 2 + ki
                                _evict(nc, qc + kb, v_all[:, kb, hv], pss[ki])

                # ---- phase Q: q up-projection + q rope ---------------
                with ExitStack() as cQ:
                    pqu = cQ.enter_context(tc.tile_pool(name="pqu", bufs=2))
                    ppair = cQ.enter_context(tc.tile_pool(name="ppair", bufs=1))
                    wqu_r = t["wqu_t"].rearrange("(b p) m -> p b m", p=128)
                    for p in range(NH // 2):
                        q_pair = ppair.tile([128, 2, L], BF16, tag="pair")
                        for piece in range(3):
                            col0 = p * 384 + piece * 128
                            wqu = pqu.tile([128, NLAT, 128], DT, tag="wqu")
                            nc.sync.dma_start(
                                out=wqu, in_=wqu_r[:, :, col0 : col0 + 128]
                            )
                            pss = [
                                ps_mm.tile([128, TOK], F32, tag="mm",
                                           name=f"psq{tk}")
                                for tk in range(NTOK)
                            ]
                            for lb in range(NLAT):
                                for tk in range(NTOK):
                                    ts = slice(tk * TOK, (tk + 1) * TOK)
                                    nc.tensor.matmul(
                                        pss[tk], wqu[:, lb], q_lat[:, lb, ts],
                                        start=(lb == 0), stop=(lb == NLAT - 1),
                                    )
                            for tk in range(NTOK):
                                ts = slice(tk * TOK, (tk + 1) * TOK)
                                if piece < 2:
                                    _evict(nc, p + piece + tk,
                                           qT_nope[:, 2 * p + piece, ts],
                                           pss[tk])
                                else:
                                    _evict(nc, p + tk, q_pair[:, 0, ts],
                                           pss[tk])
                        nc.sync.dma_start(
                            out=q_pair[0:32, 1, :], in_=q_pair[32:64, 0, :]
                        )
                        nc.sync.dma_start(
                            out=q_pair[32:64, 1, :], in_=q_pair[0:32, 0, :]
                        )
                        nc.sync.dma_start(
                            out=q_pair[64:96, 1, :], in_=q_pair[96:128, 0, :]
                        )
                        nc.sync.dma_start(
                            out=q_pair[96:128, 1, :], in_=q_pair[64:96, 0, :]
                        )
                        nc.vector.tensor_mul(
                            q_roped[:, p, :], q_pair[:, 0, :], cosf
                        )
                        nc.vector.tensor_mul(
                            q_pair[:, 0, :], q_pair[:, 1, :], sinf
                        )
                        nc.vector.tensor_add(
                            q_roped[:, p, :], q_roped[:, p, :], q_pair[:, 0, :]
                        )

                # ---- phase A: attention ------------------------------
                with ExitStack() as cA:
                    pe = cA.enter_context(tc.tile_pool(name="pe", bufs=3))
                    prd = cA.enter_context(tc.tile_pool(name="prd", bufs=2))

                    for h in range(NH):
                        hb = (h % 2) * 64
                        pr = h // 2
                        hv = slice(h * VD, (h + 1) * VD)
                        ps_vt = {}
                        ps_dt = {}
                        for qb in range(NTOK):
                            ps_vt[qb] = ps_v.tile([128, TOK], F32, tag="v", name=f"psvt{qb}")
                            ps_dt[qb] = ps_d.tile([1, TOK], F32, tag="d", name=f"psdt{qb}")
                        for kb in range(NKB):
                            k0 = kb * KB
                            ks = slice(k0, k0 + KB)
                            us = UNITS[kb]
                            sts = [
                                ps_mm.tile([128, TOK], F32, tag="mm",
                                           name=f"pss{ui}")
                                for ui in range(len(us))
                            ]
                            for (qb, qs, f, dg), st in zip(us, sts):
                                nc.tensor.matmul(
                                    st[:, :f], kv_abs[:, h, ks],
                                    qT_nope[:, h, qs : qs + f],
                                    start=True, stop=False,
                                )
                            for (qb, qs, f, dg), st in zip(us, sts):
                                nc.tensor.matmul(
                                    st[:, :f], k_roped[hb : hb + 64, ks],
                                    q_roped[hb : hb + 64, pr, qs : qs + f],
                                    start=False, stop=True,
                                )
                            ets = []
                            for (qb, qs, f, dg), st in zip(us, sts):
                                e_t = pe.tile([128, TOK], BF16, tag="e")
                                nc.scalar.activation(
                                    e_t[:, :f], st[:, :f],
                                    mybir.ActivationFunctionType.Exp,
                                    scale=SCALE,
                                )
                                if dg:
                                    nc.gpsimd.affine_select(
                                        out=e_t[:, :f], in_=e_t[:, :f],
                                        pattern=[[1, f]],
                                        compare_op=mybir.AluOpType.is_ge,
                                        fill=0.0,
                                        base=qs - k0,
                                        channel_multiplier=-1,
                                    )
                                ets.append(e_t)
                            for (qb, qs, f, dg), e_t in zip(us, ets):
                                lo = qs - qb * TOK
                                nc.tensor.matmul(
                                    ps_dt[qb][:, lo : lo + f], ones_col_b,
                                    e_t[:, :f],
                                    start=(kb == FIRST_KB[qb]),
                                    stop=(kb == LAST_KB[qb]),
                                )
                            for (qb, qs, f, dg), e_t in zip(us, ets):
                                lo = qs - qb * TOK
                                nc.tensor.matmul(
                                    ps_vt[qb][:, lo : lo + f],
                                    v_all[:, kb, hv], e_t[:, :f],
                                    start=(kb == FIRST_KB[qb]),
                                    stop=(kb == LAST_KB[qb]),
                                )
                            for qb in range(NTOK):
                                if kb == LAST_KB[qb]:
                                    ts = slice(qb * TOK, (qb + 1) * TOK)
                                    rd = prd.tile([1, TOK], F32, tag="rd")
                                    nc.vector.reciprocal_approx_fast(
                                        out=rd, in_=ps_dt[qb]
                                    )
                                    rb = prd.tile([128, TOK], F32, tag="rb")
                                    nc.gpsimd.partition_broadcast(rb, rd)
                                    nc.vector.tensor_mul(
                                        vn[:, h, ts], ps_vt[qb], rb
                                    )
                        if h == 6:
                            fetch_wout(0)
                            fetch_wout(1)

            # ---- phase O: output projection (cM closed) --------------
            with ExitStack() as cO:
                po = cO.enter_context(tc.tile_pool(name="po", bufs=3))
                for c in range(16):
                    if wouts[c] is None:
                        fetch_wout(c)
                    row = c * 128
                    pss = [
                        ps_mm.tile([128, TOK], F32, tag="mm",
                                   name=f"pso{tk}")
                        for tk in range(NTOK)
                    ]
                    for hbk in range(NH):
                        for tk in range(NTOK):
                            ts = slice(tk * TOK, (tk + 1) * TOK)
                            nc.tensor.matmul(
                                pss[tk], wouts[c][:, hbk, :], vn[:, hbk, ts],
                                start=(hbk == 0), stop=(hbk == NH - 1),
                            )
                    for tk in range(NTOK):
                        ts = slice(tk * TOK, (tk + 1) * TOK)
                        o_t = po.tile([128, TOK], F32, tag="o")
                        _evict(nc, c + tk, o_t, pss[tk])
                        nc.sync.dma_start(
                            out=t["outT"][row : row + 128, ts], in_=o_t
                        )


# ======================================================================
# host side
# ======================================================================

_NC_CACHE = {}


def _get_nc():
    if "nc" not in _NC_CACHE:
        _NC_CACHE["nc"] = build_nc()
    return _NC_CACHE["nc"]


def _prep_shared(inputs):
    wq_down = np.asarray(inputs["Wq_down"], np.float32)
    wq_up = np.asarray(inputs["Wq_up"], np.float32)
    wkv_down = np.asarray(inputs["Wkv_down"], np.float32)
    wkv_up = np.asarray(inputs["Wkv_up"], np.float32)
    wout = np.asarray(inputs["Wout"], np.float32)
    rms_q_w = np.asarray(inputs["rms_q_w"], np.float32)
    rms_kv_w = np.asarray(inputs["rms_kv_w"], np.float32)
    freq = np.asarray(inputs["freq_cis"], np.float32)  # [L, 32, 2]

    # split re/im layout for all rope dims: re parts first, then im parts
    rope_perm = np.concatenate(
        [np.arange(0, ROPE, 2), np.arange(1, ROPE, 2)]
    )  # [64]

    # combined down-proj: q latent | kv latent | k-rope (re/im split), pad
    wd = np.zeros((1664, D), np.float32)
    wd[:DQ] = wq_down
    wd[DQ : DQ + KVR] = wkv_down[:KVR]
    wd[DQ + KVR : DQ + KVR + ROPE] = wkv_down[KVR:][rope_perm]
    wd_t = np.ascontiguousarray(wd.T).astype(ml_dtypes.bfloat16)  # [D, 1664]

    # rope tables (dim-major, split re/im, duplicated partition halves)
    cos = freq[:, :, 0].T  # [32, L]
    sin = freq[:, :, 1].T
    cosf64 = np.vstack([cos, cos])  # [64, L]
    sinf64 = np.vstack([-sin, sin])
    cosf = np.ascontiguousarray(np.vstack([cosf64, cosf64]))  # [128, L]
    sinf = np.ascontiguousarray(np.vstack([sinf64, sinf64]))

    wq_up3 = (wq_up * rms_q_w[None, :]).reshape(H, HD, DQ)
    wq_nope = wq_up3[:, :NOPE, :]                      # [H, 128, DQ]
    wq_rope = wq_up3[:, NOPE:, :][:, rope_perm, :]     # [H, 64, DQ]
    wkv_up3 = wkv_up.reshape(H, NOPE + VD, KVR)
    wout3 = wout.reshape(D, H, VD)

    per_g = []
    for g in range(2):
        hs = list(range(g * NH, (g + 1) * NH))
        # q up: per pair [nope(2p) | nope(2p+1) | rope(2p)+rope(2p+1)]
        cols = []
        for p in range(NH // 2):
            h0, h1 = hs[2 * p], hs[2 * p + 1]
            cols.append(wq_nope[h0])
            cols.append(wq_nope[h1])
            cols.append(wq_rope[h0])
            cols.append(wq_rope[h1])
        wqu_t = np.ascontiguousarray(
            np.concatenate(cols, axis=0).T
        )  # [DQ, 1536]

        wabs = wkv_up3[hs, :NOPE, :] * rms_kv_w[None, None, :]  # [8,128,512]
        # per head: [KVR, NOPE] -> [128, 4, 128]
        wabs_t = np.ascontiguousarray(
            wabs.transpose(0, 2, 1).reshape(NH, NKV, 128, NOPE)
            .transpose(0, 2, 1, 3)
        )  # [8, 128, 4, 128]

        wv = wkv_up3[hs, NOPE:, :] * rms_kv_w[None, None, :]  # [8, 128, 512]
        # [KVR, NH*VD] -> [128, 4, 1024]
        wv_t = np.ascontiguousarray(
            wv.transpose(2, 0, 1).reshape(NKV, 128, NH * VD)
            .transpose(1, 0, 2)
        )  # [128, 4, 1024]

        wout_t = np.ascontiguousarray(
            wout3[:, hs, :].transpose(1, 2, 0).reshape(NH * VD, D)
        ).astype(ml_dtypes.bfloat16)  # [1024, 2048]
        per_g.append(
            {
                "wd_t": wd_t,
                "wqu_t": wqu_t,
                "wabs_t": wabs_t,
                "wv_t": wv_t,
                "wout_t": wout_t,
                "cosf": cosf,
                "sinf": sinf,
            }
        )
    return per_g


def make_in_maps(inputs):
    x = np.asarray(inputs["x"], np.float32)
    per_g = _prep_shared(inputs)
    in_maps = []
    for c in range(N_CORES):
        b, g = c // 2, c % 2
        m = dict(per_g[g])
        m["x_t"] = np.ascontiguousarray(x[b].T).astype(ml_dtypes.bfloat16)
        in_maps.append(m)
    return in_maps


def kernel(**inputs):
    nc = _get_nc()
    in_maps = make_in_maps(inputs)
    res = bass_utils.run_bass_kernel_spmd(
        nc, in_maps, core_ids=list(range(N_CORES))
    ).results
    out = np.empty((B, L, D), np.float32)
    for b in range(B):
        out[b] = (res[2 * b]["outT"] + res[2 * b + 1]["outT"]).T
    return out


# revision 16
# speedup vs baseline: 2.6246x; 1.1372x over previous
"""MLA prefill kernel for Trainium2, 8 NeuronCores.

Sharding: core c -> (batch b = c // 2, head-group g = c % 2). Each core
computes its batch's full sequence for its 8 heads, producing a partial
output (transposed, [2048, 1024]); the host sums the two head-group
partials per batch and transposes back.

Layout strategy (all on-chip matmuls contract over the partition dim):
  x arrives transposed ([D, L]) per batch, streamed in two d-halves;
  down projections produce latent-major activations.  The kv_up
  nope-slice is absorbed into the KV side: per head kv_abs[np, k] =
  Wabs_h^T kv_lat and V[k, vd] = kv_lat^T Wv_h are precomputed once, so
  each attention score tile is one 128-contraction matmul (+ a 64-wide
  rope matmul) and each value tile is a single matmul per key block.
  Attention runs "k-major" (scores^T [key, query]) with ideal causal
  packing: per key block only the valid query range is computed (F
  rounded up to >=256 to keep fp32r matmuls at full rate).  Softmax
  max-subtraction is skipped (scores are O(1)).  Diagonal blocks are
  masked post-exp with affine_select; denominators via ones-column
  matmuls; reciprocals via the fast DVE approximation.  The score path
  stays fp32r; the value path (V, probs, vn, Wout) is bf16 to fit SBUF
  (vn never round-trips through DRAM).
"""

import math
from contextlib import ExitStack

import ml_dtypes
import numpy as np

import concourse.bass as bass
import concourse.mybir as mybir
import concourse.tile as tile
from concourse import bacc, bass_utils

# ---- problem constants -------------------------------------------------
B, L, D = 4, 1024, 2048
H, NOPE, ROPE, VD, KVR = 16, 128, 64, 128, 512
DQ = 1024            # q latent dim
HD = NOPE + ROPE     # 192 per-head q dim
EPS = 1e-6
NH = 8               # heads per core
N_CORES = 8
SCALE = 1.0 / math.sqrt(NOPE + ROPE)

F32 = mybir.dt.float32
F32R = mybir.dt.float32r
BF16 = mybir.dt.bfloat16
DT = F32R

TOK = 512            # q-tile / PSUM bank width
NTOK = L // TOK      # 2
KB = 128             # key-token block
NKB = L // KB        # 8
ND = D // 128        # 16 contraction blocks over model dim
NLAT = DQ // 128     # 8 blocks over q latent
NKV = KVR // 128     # 4 blocks over kv latent
# down-proj output blocks: 8 q, 4 kv, 1 rope(64); kv+rope emitted first
OB_ORDER = [8, 9, 10, 11, 12] + list(range(8))


def _unit_table():
    """Per kb: list of (qb, q_start, F, diag) covering the valid causal
    query range, F rounded up to >=256 for full-rate fp32r matmuls."""
    units = {}
    for kb in range(NKB):
        k0 = kb * KB
        lst = []
        for qb in range(NTOK):
            q0 = qb * TOK
            if k0 >= q0 + TOK:
                continue  # fully masked
            if k0 + KB <= q0:
                lst.append((qb, q0, TOK, False))  # full block
            else:
                qs, f = k0, q0 + TOK - k0
                if f < 256:
                    qs, f = qs - (256 - f), 256
                lst.append((qb, qs, f, True))
        units[kb] = lst
    return units


UNITS = _unit_table()
FIRST_KB = {0: 0, 1: 0}
LAST_KB = {0: 3, 1: 7}


def build_nc():
    nc = bacc.Bacc("TRN2", target_bir_lowering=False, debug=False)

    t = {}
    t["x_t"] = nc.dram_tensor("x_t", [D, L], BF16, kind="ExternalInput").ap()
    t["wd_t"] = nc.dram_tensor("wd_t", [D, 1664], BF16, kind="ExternalInput").ap()
    t["wqu_t"] = nc.dram_tensor("wqu_t", [DQ, NH * HD], DT, kind="ExternalInput").ap()
    t["wabs_t"] = nc.dram_tensor(
        "wabs_t", [NH, 128, NKV, NOPE], DT, kind="ExternalInput"
    ).ap()
    t["wv_t"] = nc.dram_tensor(
        "wv_t", [128, NKV, NH * VD], DT, kind="ExternalInput"
    ).ap()
    t["wout_t"] = nc.dram_tensor(
        "wout_t", [NH * VD, D], BF16, kind="ExternalInput"
    ).ap()
    t["cosf"] = nc.dram_tensor("cosf", [128, L], F32, kind="ExternalInput").ap()
    t["sinf"] = nc.dram_tensor("sinf", [128, L], F32, kind="ExternalInput").ap()
    t["outT"] = nc.dram_tensor("outT", [D, L], F32, kind="ExternalOutput").ap()

    with tile.TileContext(nc) as tc:
        _emit(tc, t)
    nc.compile()
    return nc


def _evict(nc, i, dst, src):
    """PSUM -> SBUF eviction, alternating engines to balance load."""
    if i % 2 == 0:
        nc.scalar.copy(dst, src)
    else:
        nc.vector.tensor_copy(dst, src)


def _rms_stats(tc, ps_d, prow, psq, ones_col_r, eps_t, lat, nlb, dim, r_row):
    """Compute per-token reciprocal RMS of `lat` ([128, nlb, L]) into
    r_row ([1, L], fp32).  No scaling is applied here — the scale is
    folded into downstream evictions so matmuls never wait on it."""
    nc = tc.nc
    for tk in range(NTOK):
        ts = slice(tk * TOK, (tk + 1) * TOK)
        ps_ssq = ps_d.tile([1, TOK], F32, tag="d")
        for lb in range(nlb):
            sq = psq.tile([128, TOK], DT, tag="sq")
            sl = lat[:, lb, ts]
            nc.vector.tensor_mul(sq, sl, sl)
            nc.tensor.matmul(
                ps_ssq, ones_col_r, sq, start=(lb == 0), stop=(lb == nlb - 1)
            )
        rt = prow.tile([1, TOK], F32, tag="rt")
        nc.scalar.activation(
            rt, ps_ssq, mybir.ActivationFunctionType.Sqrt,
            bias=eps_t, scale=1.0 / dim,
        )
        nc.vector.reciprocal_approx_fast(out=r_row[:, ts], in_=rt)


def _emit(tc, t):
    nc = tc.nc
    with ExitStack() as c0:
        c0.enter_context(
            nc.allow_low_precision(reason="fp32r/bf16 rounding is intentional")
        )
        from concourse import library_config

        nc.gpsimd.load_library(library_config.attnmlp)

        glob = c0.enter_context(tc.tile_pool(name="glob", bufs=1))
        ps_mm = c0.enter_context(tc.tile_pool(name="ps_mm", bufs=4, space="PSUM"))
        ps_v = c0.enter_context(tc.tile_pool(name="ps_v", bufs=2, space="PSUM"))
        ps_d = c0.enter_context(tc.tile_pool(name="ps_d", bufs=2, space="PSUM"))

        # ---- constants -----------------------------------------------
        ones_f32 = glob.tile([128, 128], F32, tag="ones32")
        nc.vector.memset(ones_f32, 1.0)
        ones_col_r = glob.tile([128, 1], DT, tag="onesr_c")
        nc.vector.tensor_copy(ones_col_r, ones_f32[:, :1])
        ones_col_b = glob.tile([128, 1], BF16, tag="onesb_c")
        nc.vector.tensor_copy(ones_col_b, ones_f32[:, :1])
        eps_t = glob.tile([1, 1], F32, tag="eps")
        nc.vector.memset(eps_t, EPS)
        k_roped = glob.tile([128, L], BF16, tag="kroped")
        v_all = glob.tile([128, NKB, NH * VD], BF16, tag="vall")

        with ExitStack() as cL:
            pL = cL.enter_context(tc.tile_pool(name="pL", bufs=1))
            cosf = pL.tile([128, L], F32, tag="cosf")
            nc.sync.dma_start(out=cosf, in_=t["cosf"])
            sinf = pL.tile([128, L], F32, tag="sinf")
            nc.sync.dma_start(out=sinf, in_=t["sinf"])
            kv_lat = pL.tile([128, NKV, L], DT, tag="kvlat")
            q_lat = pL.tile([128, NLAT, L], DT, tag="qlat")
            rkv_row = pL.tile([1, L], F32, tag="rkvrow")
            rq_row = pL.tile([1, L], F32, tag="rqrow")
            rkv_b = pL.tile([128, L], F32, tag="rkvb")
            rq_b = pL.tile([128, L], F32, tag="rqb")

            # ---- phase X: down projections (x in two d-halves) -------
            with ExitStack() as cX:
                px = cX.enter_context(tc.tile_pool(name="px", bufs=1))
                pwd = cX.enter_context(tc.tile_pool(name="pwd", bufs=3))
                prow = cX.enter_context(tc.tile_pool(name="prow", bufs=1))
                psq = cX.enter_context(tc.tile_pool(name="psq", bufs=3))
                pkr = cX.enter_context(tc.tile_pool(name="pkr", bufs=1))

                x_r = t["x_t"].rearrange("(b p) t -> p b t", p=128)
                wd_r = t["wd_t"].rearrange("(b p) m -> p b m", p=128)
                kr_pair = pkr.tile([128, 2, L], BF16, tag="krpair")

                warm_f = pkr.tile([128, TOK], F32, tag="warmf")
                nc.vector.memset(warm_f, 0.0)
                warm = pkr.tile([128, TOK], DT, tag="warm")
                nc.vector.tensor_copy(warm, warm_f)
                for _ in range(56):
                    ps = ps_mm.tile([128, TOK], F32, tag="mm")
                    nc.tensor.matmul(ps, warm[:, :128], warm)

                x_sb = px.tile([128, ND, L], BF16, tag="x")
                for ch in range(8):
                    nc.sync.dma_start(
                        out=x_sb[:, ch * 2 : ch * 2 + 2, :],
                        in_=x_r[:, ch * 2 : ch * 2 + 2, :],
                    )
                for ob in OB_ORDER:
                    cw = 64 if ob == 12 else 128
                    wd = pwd.tile([128, ND, 128], BF16, tag="wd")
                    nc.sync.dma_start(
                        out=wd[:, :, :cw],
                        in_=wd_r[:, :, ob * 128 : ob * 128 + cw],
                    )
                    pss = [
                        ps_mm.tile([128, TOK], F32, tag="mm",
                                   name=f"psd{tk}")
                        for tk in range(NTOK)
                    ]
                    for db in range(ND):
                        for tk in range(NTOK):
                            ts = slice(tk * TOK, (tk + 1) * TOK)
                            nc.tensor.matmul(
                                pss[tk][:cw], wd[:, db, :cw], x_sb[:, db, ts],
                                start=(db == 0), stop=(db == ND - 1),
                            )
                    for tk in range(NTOK):
                        ts = slice(tk * TOK, (tk + 1) * TOK)
                        if ob < 8:
                            dst = q_lat[:, ob, ts]
                        elif ob < 12:
                            dst = kv_lat[:, ob - 8, ts]
                        else:
                            dst = kr_pair[:64, 0, ts]
                        _evict(nc, ob + tk, dst, pss[tk][:cw])

                    if ob == 12:
                        # kv RMS stats + k rope (overlaps q blocks)
                        _rms_stats(tc, ps_d, prow, psq, ones_col_r,
                                   eps_t, kv_lat, NKV, KVR, rkv_row)
                        nc.gpsimd.partition_broadcast(rkv_b, rkv_row)
                        for lb in range(NKV):
                            nc.vector.tensor_mul(
                                kv_lat[:, lb, :], kv_lat[:, lb, :], rkv_b
                            )
                        nc.sync.dma_start(
                            out=kr_pair[0:32, 1, :], in_=kr_pair[32:64, 0, :]
                        )
                        nc.sync.dma_start(
                            out=kr_pair[32:64, 1, :], in_=kr_pair[0:32, 0, :]
                        )
                        nc.vector.tensor_mul(
                            k_roped[0:64], kr_pair[0:64, 0, :], cosf[0:64]
                        )
                        nc.vector.tensor_mul(
                            kr_pair[0:64, 0, :], kr_pair[0:64, 1, :],
                            sinf[0:64],
                        )
                        nc.vector.tensor_add(
                            k_roped[0:64], k_roped[0:64], kr_pair[0:64, 0, :]
                        )
                        nc.sync.dma_start(
                            out=k_roped[64:128], in_=k_roped[0:64]
                        )

                # q RMS stats (resolve while kv_abs/V matmuls run)
                _rms_stats(tc, ps_d, prow, psq, ones_col_r,
                           eps_t, q_lat, NLAT, DQ, rq_row)
                nc.gpsimd.partition_broadcast(rq_b, rq_row)

            # ---- phases P/Q/A under attention-lived pools ------------
            pwo = cL.enter_context(tc.tile_pool(name="pwo", bufs=4))
            pvn = cL.enter_context(tc.tile_pool(name="pvn", bufs=1))
            vn = pvn.tile([128, NH, L], BF16, tag="vn")
            wout_r = t["wout_t"].rearrange("(b p) m -> p b m", p=128)
            wouts = [None] * 16

            def fetch_wout(c):
                wouts[c] = pwo.tile([128, NH, 128], BF16, tag="wout", name=f"wout{c}")
                nc.sync.dma_start(
                    out=wouts[c], in_=wout_r[:, :, c * 128 : (c + 1) * 128]
                )

            with ExitStack() as cM:
                pM = cM.enter_context(tc.tile_pool(name="pM", bufs=1))
                kv_abs = pM.tile([128, NH, L], DT, tag="kvabs")
                qT_nope = pM.tile([128, NH, L], DT, tag="qnope")
                q_roped = pM.tile([128, NH // 2, L], BF16, tag="qroped")

                # ---- phase P: kv_abs + V precompute ------------------
                with ExitStack() as cP:
                    pw = cP.enter_context(tc.tile_pool(name="pw", bufs=2))
                    for h in range(NH):
                        wabs = pw.tile([128, NKV, NOPE], DT, tag="wabs")
                        nc.sync.dma_start(out=wabs, in_=t["wabs_t"][h])
                        pss = [
                            ps_mm.tile([128, TOK], F32, tag="mm",
                                       name=f"psp{tk}")
                            for tk in range(NTOK)
                        ]
                        for lb in range(NKV):
                            for tk in range(NTOK):
                                ts = slice(tk * TOK, (tk + 1) * TOK)
                                nc.tensor.matmul(
                                    pss[tk], wabs[:, lb], kv_lat[:, lb, ts],
                                    start=(lb == 0), stop=(lb == NKV - 1),
                                )
                        for tk in range(NTOK):
                            ts = slice(tk * TOK, (tk + 1) * TOK)
                            _evict(nc, h + tk, kv_abs[:, h, ts], pss[tk])

                    for qc in range(2):
                        hv = slice(qc * 512, (qc + 1) * 512)
                        wv = pw.tile([128, NKV, 512], BF16, tag="wv")
                        nc.sync.dma_start(out=wv, in_=t["wv_t"][:, :, hv])
                        for kp in range(NKB // 2):
                            pss = [
                                ps_mm.tile([128, 512], F32, tag="mm",
                                           name=f"psv{ki}")
                                for ki in range(2)
                            ]
                            for lb in range(NKV):
                                for ki in range(2):
                                    kb = kp * 2 + ki
                                    ks = slice(kb * KB, (kb + 1) * KB)
                                    nc.tensor.matmul(
                                        pss[ki], kv_lat[:, lb, ks],
                                        wv[:, lb, :],
                                        start=(lb == 0), stop=(lb == NKV - 1),
                                    )
                            for ki in range(2):
                                kb = kp * 2 + ki
                                _evict(nc, qc + kp + ki, v_all[:, kb, hv],
                                       pss[ki])

                # ---- phase Q: q up-projection + q rope ---------------
                with ExitStack() as cQ:
                    pqu = cQ.enter_context(tc.tile_pool(name="pqu", bufs=3))
                    ppair = cQ.enter_context(tc.tile_pool(name="ppair", bufs=1))
                    wqu_r = t["wqu_t"].rearrange("(b p) m -> p b m", p=128)
                    for p in range(NH // 2):
                        q_pair = ppair.tile([128, 2, L], BF16, tag="pair")
                        for piece in range(3):
                            col0 = p * 384 + piece * 128
                            wqu = pqu.tile([128, NLAT, 128], DT, tag="wqu")
                            nc.sync.dma_start(
                                out=wqu, in_=wqu_r[:, :, col0 : col0 + 128]
                            )
                            pss = [
                                ps_mm.tile([128, TOK], F32, tag="mm",
                                           name=f"psq{tk}")
                                for tk in range(NTOK)
                            ]
                            for lb in range(NLAT):
                                for tk in range(NTOK):
                                    ts = slice(tk * TOK, (tk + 1) * TOK)
                                    nc.tensor.matmul(
                                        pss[tk], wqu[:, lb], q_lat[:, lb, ts],
                                        start=(lb == 0), stop=(lb == NLAT - 1),
                                    )
                            for tk in range(NTOK):
                                ts = slice(tk * TOK, (tk + 1) * TOK)
                                if piece < 2:
                                    nc.vector.tensor_mul(
                                        qT_nope[:, 2 * p + piece, ts],
                                        pss[tk], rq_b[:, ts],
                                    )
                                else:
                                    nc.vector.tensor_mul(
                                        q_pair[:, 0, ts], pss[tk], rq_b[:, ts]
                                    )
                        nc.sync.dma_start(
                            out=q_pair[0:32, 1, :], in_=q_pair[32:64, 0, :]
                        )
                        nc.sync.dma_start(
                            out=q_pair[32:64, 1, :], in_=q_pair[0:32, 0, :]
                        )
                        nc.sync.dma_start(
                            out=q_pair[64:96, 1, :], in_=q_pair[96:128, 0, :]
                        )
                        nc.sync.dma_start(
                            out=q_pair[96:128, 1, :], in_=q_pair[64:96, 0, :]
                        )
                        nc.vector.tensor_mul(
                            q_roped[:, p, :], q_pair[:, 0, :], cosf
                        )
                        nc.vector.tensor_mul(
                            q_pair[:, 0, :], q_pair[:, 1, :], sinf
                        )
                        nc.vector.tensor_add(
                            q_roped[:, p, :], q_roped[:, p, :], q_pair[:, 0, :]
                        )

                # ---- phase A: attention ------------------------------
                with ExitStack() as cA:
                    pe = cA.enter_context(tc.tile_pool(name="pe", bufs=5))
                    prd = cA.enter_context(tc.tile_pool(name="prd", bufs=2))

                    for h in range(NH):
                        hb = (h % 2) * 64
                        pr = h // 2
                        hv = slice(h * VD, (h + 1) * VD)
                        ps_vt = {}
                        ps_dt = {}
                        for qb in range(NTOK):
                            ps_vt[qb] = ps_v.tile([128, TOK], F32, tag="v", name=f"psvt{qb}")
                            ps_dt[qb] = ps_d.tile([1, TOK], F32, tag="d", name=f"psdt{qb}")
                        for kb in range(NKB):
                            k0 = kb * KB
                            ks = slice(k0, k0 + KB)
                            us = UNITS[kb]
                            sts = [
                                ps_mm.tile([128, TOK], F32, tag="mm",
                                           name=f"pss{ui}")
                                for ui in range(len(us))
                            ]
                            for (qb, qs, f, dg), st in zip(us, sts):
                                nc.tensor.matmul(
                                    st[:, :f], kv_abs[:, h, ks],
                                    qT_nope[:, h, qs : qs + f],
                                    start=True, stop=False,
                                )
                            for (qb, qs, f, dg), st in zip(us, sts):
                                nc.tensor.matmul(
                                    st[:, :f], k_roped[hb : hb + 64, ks],
                                    q_roped[hb : hb + 64, pr, qs : qs + f],
                                    start=False, stop=True,
                                )
                            ets = []
                            for (qb, qs, f, dg), st in zip(us, sts):
                                e_t = pe.tile([128, TOK], BF16, tag="e")
                                nc.scalar.activation(
                                    e_t[:, :f], st[:, :f],
                                    mybir.ActivationFunctionType.Exp,
                                    scale=SCALE,
                                )
                                if dg:
                                    nc.gpsimd.affine_select(
                                        out=e_t[:, :f], in_=e_t[:, :f],
                                        pattern=[[1, f]],
                                        compare_op=mybir.AluOpType.is_ge,
                                        fill=0.0,
                                        base=qs - k0,
                                        channel_multiplier=-1,
                                    )
                                ets.append(e_t)
                            for (qb, qs, f, dg), e_t in zip(us, ets):
                                lo = qs - qb * TOK
                                nc.tensor.matmul(
                                    ps_dt[qb][:, lo : lo + f], ones_col_b,
                                    e_t[:, :f],
                                    start=(kb == FIRST_KB[qb]),
                                    stop=(kb == LAST_KB[qb]),
                                )
                            for (qb, qs, f, dg), e_t in zip(us, ets):
                                lo = qs - qb * TOK
                                nc.tensor.matmul(
                                    ps_vt[qb][:, lo : lo + f],
                                    v_all[:, kb, hv], e_t[:, :f],
                                    start=(kb == FIRST_KB[qb]),
                                    stop=(kb == LAST_KB[qb]),
                                )
                            for qb in range(NTOK):
                                if kb == LAST_KB[qb]:
                                    ts = slice(qb * TOK, (qb + 1) * TOK)
                                    rd = prd.tile([1, TOK], F32, tag="rd")
                                    nc.vector.reciprocal_approx_fast(
                                        out=rd, in_=ps_dt[qb]
                                    )
                                    rb = prd.tile([128, TOK], F32, tag="rb")
                                    nc.gpsimd.partition_broadcast(rb, rd)
                                    nc.vector.tensor_mul(
                                        vn[:, h, ts], ps_vt[qb], rb
                                    )
                        if h == 6:
                            fetch_wout(0)
                            fetch_wout(1)

            # ---- phase O: output projection (cM closed) --------------
            with ExitStack() as cO:
                po = cO.enter_context(tc.tile_pool(name="po", bufs=4))
                for c in range(16):
                    if wouts[c] is None:
                        fetch_wout(c)
                    row = c * 128
                    pss = [
                        ps_mm.tile([128, TOK], F32, tag="mm",
                                   name=f"pso{tk}")
                        for tk in range(NTOK)
                    ]
                    for hbk in range(NH):
                        for tk in range(NTOK):
                            ts = slice(tk * TOK, (tk + 1) * TOK)
                            nc.tensor.matmul(
                                pss[tk], wouts[c][:, hbk, :], vn[:, hbk, ts],
                                start=(hbk == 0), stop=(hbk == NH - 1),
                            )
                    for tk in range(NTOK):
                        ts = slice(tk * TOK, (tk + 1) * TOK)
                        o_t = po.tile([128, TOK], F32, tag="o")
                        _evict(nc, c + tk, o_t, pss[tk])
                        nc.sync.dma_start(
                            out=t["outT"][row : row + 128, ts], in_=o_t
                        )


# ======================================================================
# host side
# ======================================================================

_NC_CACHE = {}


def _get_nc():
    if "nc" not in _NC_CACHE:
        _NC_CACHE["nc"] = build_nc()
    return _NC_CACHE["nc"]


def _prep_shared(inputs):
    wq_down = np.asarray(inputs["Wq_down"], np.float32)
    wq_up = np.asarray(inputs["Wq_up"], np.float32)
    wkv_down = np.asarray(inputs["Wkv_down"], np.float32)
    wkv_up = np.asarray(inputs["Wkv_up"], np.float32)
    wout = np.asarray(inputs["Wout"], np.float32)
    rms_q_w = np.asarray(inputs["rms_q_w"], np.float32)
    rms_kv_w = np.asarray(inputs["rms_kv_w"], np.float32)
    freq = np.asarray(inputs["freq_cis"], np.float32)  # [L, 32, 2]

    # split re/im layout for all rope dims: re parts first, then im parts
    rope_perm = np.concatenate(
        [np.arange(0, ROPE, 2), np.arange(1, ROPE, 2)]
    )  # [64]

    # combined down-proj: q latent | kv latent | k-rope (re/im split), pad
    wd = np.zeros((1664, D), np.float32)
    wd[:DQ] = wq_down
    wd[DQ : DQ + KVR] = wkv_down[:KVR]
    wd[DQ + KVR : DQ + KVR + ROPE] = wkv_down[KVR:][rope_perm]
    wd_t = np.ascontiguousarray(wd.T).astype(ml_dtypes.bfloat16)  # [D, 1664]

    # rope tables (dim-major, split re/im, duplicated partition halves)
    cos = freq[:, :, 0].T  # [32, L]
    sin = freq[:, :, 1].T
    cosf64 = np.vstack([cos, cos])  # [64, L]
    sinf64 = np.vstack([-sin, sin])
    cosf = np.ascontiguousarray(np.vstack([cosf64, cosf64]))  # [128, L]
    sinf = np.ascontiguousarray(np.vstack([sinf64, sinf64]))

    wq_up3 = (wq_up * rms_q_w[None, :]).reshape(H, HD, DQ)
    wq_nope = wq_up3[:, :NOPE, :]                      # [H, 128, DQ]
    wq_rope = wq_up3[:, NOPE:, :][:, rope_perm, :]     # [H, 64, DQ]
    wkv_up3 = wkv_up.reshape(H, NOPE + VD, KVR)
    wout3 = wout.reshape(D, H, VD)

    per_g = []
    for g in range(2):
        hs = list(range(g * NH, (g + 1) * NH))
        # q up: per pair [nope(2p) | nope(2p+1) | rope(2p)+rope(2p+1)]
        cols = []
        for p in range(NH // 2):
            h0, h1 = hs[2 * p], hs[2 * p + 1]
            cols.append(wq_nope[h0])
            cols.append(wq_nope[h1])
            cols.append(wq_rope[h0])
            cols.append(wq_rope[h1])
        wqu_t = np.ascontiguousarray(
            np.concatenate(cols, axis=0).T
        )  # [DQ, 1536]

        wabs = wkv_up3[hs, :NOPE, :] * rms_kv_w[None, None, :]  # [8,128,512]
        # per head: [KVR, NOPE] -> [128, 4, 128]
        wabs_t = np.ascontiguousarray(
            wabs.transpose(0, 2, 1).reshape(NH, NKV, 128, NOPE)
            .transpose(0, 2, 1, 3)
        )  # [8, 128, 4, 128]

        wv = wkv_up3[hs, NOPE:, :] * rms_kv_w[None, None, :]  # [8, 128, 512]
        # [KVR, NH*VD] -> [128, 4, 1024]
        wv_t = np.ascontiguousarray(
            wv.transpose(2, 0, 1).reshape(NKV, 128, NH * VD)
            .transpose(1, 0, 2)
        )  # [128, 4, 1024]

        wout_t = np.ascontiguousarray(
            wout3[:, hs, :].transpose(1, 2, 0).reshape(NH * VD, D)
        ).astype(ml_dtypes.bfloat16)  # [1024, 2048]
        per_g.append(
            {
                "wd_t": wd_t,
                "wqu_t": wqu_t,
                "wabs_t": wabs_t,
                "wv_t": wv_t,
                "wout_t": wout_t,
                "cosf": cosf,
                "sinf": sinf,
            }
        )
    return per_g


def make_in_maps(inputs):
    x = np.asarray(inputs["x"], np.float32)
    per_g = _prep_shared(inputs)
    in_maps = []
    for c in range(N_CORES):
        b, g = c // 2, c % 2
        m = dict(per_g[g])
        m["x_t"] = np.ascontiguousarray(x[b].T).astype(ml_dtypes.bfloat16)
        in_maps.append(m)
    return in_maps


def kernel(**inputs):
    nc = _get_nc()
    in_maps = make_in_maps(inputs)
    res = bass_utils.run_bass_kernel_spmd(
        nc, in_maps, core_ids=list(range(N_CORES))
    ).results
    out = np.empty((B, L, D), np.float32)
    for b in range(B):
        out[b] = (res[2 * b]["outT"] + res[2 * b + 1]["outT"]).T
    return out


# revision 17
# speedup vs baseline: 2.6253x; 1.0003x over previous
"""MLA prefill kernel for Trainium2, 8 NeuronCores.

Sharding: core c -> (batch b = c // 2, head-group g = c % 2). Each core
computes its batch's full sequence for its 8 heads, producing a partial
output (transposed, [2048, 1024]); the host sums the two head-group
partials per batch and transposes back.

Layout strategy (all on-chip matmuls contract over the partition dim):
  x arrives transposed ([D, L]) per batch, streamed in two d-halves;
  down projections produce latent-major activations.  The kv_up
  nope-slice is absorbed into the KV side: per head kv_abs[np, k] =
  Wabs_h^T kv_lat and V[k, vd] = kv_lat^T Wv_h are precomputed once, so
  each attention score tile is one 128-contraction matmul (+ a 64-wide
  rope matmul) and each value tile is a single matmul per key block.
  Attention runs "k-major" (scores^T [key, query]) with ideal causal
  packing: per key block only the valid query range is computed (F
  rounded up to >=256 to keep fp32r matmuls at full rate).  Softmax
  max-subtraction is skipped (scores are O(1)).  Diagonal blocks are
  masked post-exp with affine_select; denominators via ones-column
  matmuls; reciprocals via the fast DVE approximation.  The score path
  stays fp32r; the value path (V, probs, vn, Wout) is bf16 to fit SBUF
  (vn never round-trips through DRAM).
"""

import math
from contextlib import ExitStack

import ml_dtypes
import numpy as np

import concourse.bass as bass
import concourse.mybir as mybir
import concourse.tile as tile
from concourse import bacc, bass_utils

# ---- problem constants -------------------------------------------------
B, L, D = 4, 1024, 2048
H, NOPE, ROPE, VD, KVR = 16, 128, 64, 128, 512
DQ = 1024            # q latent dim
HD = NOPE + ROPE     # 192 per-head q dim
EPS = 1e-6
NH = 8               # heads per core
N_CORES = 8
SCALE = 1.0 / math.sqrt(NOPE + ROPE)

F32 = mybir.dt.float32
F32R = mybir.dt.float32r
BF16 = mybir.dt.bfloat16
DT = F32R

TOK = 512            # q-tile / PSUM bank width
NTOK = L // TOK      # 2
KB = 128             # key-token block
NKB = L // KB        # 8
ND = D // 128        # 16 contraction blocks over model dim
NLAT = DQ // 128     # 8 blocks over q latent
NKV = KVR // 128     # 4 blocks over kv latent
# down-proj output blocks: 8 q, 4 kv, 1 rope(64); kv+rope emitted first
OB_ORDER = [8, 9, 10, 11, 12] + list(range(8))


def _unit_table():
    """Per kb: list of (qb, q_start, F, diag) covering the valid causal
    query range, F rounded up to >=256 for full-rate fp32r matmuls."""
    units = {}
    for kb in range(NKB):
        k0 = kb * KB
        lst = []
        for qb in range(NTOK):
            q0 = qb * TOK
            if k0 >= q0 + TOK:
                continue  # fully masked
            if k0 + KB <= q0:
                lst.append((qb, q0, TOK, False))  # full block
            else:
                qs, f = k0, q0 + TOK - k0
                if f < 256:
                    qs, f = qs - (256 - f), 256
                lst.append((qb, qs, f, True))
        units[kb] = lst
    return units


UNITS = _unit_table()
FIRST_KB = {0: 0, 1: 0}
LAST_KB = {0: 3, 1: 7}


def build_nc():
    nc = bacc.Bacc("TRN2", target_bir_lowering=False, debug=False)

    t = {}
    t["x_t"] = nc.dram_tensor("x_t", [D, L], BF16, kind="ExternalInput").ap()
    t["wd_t"] = nc.dram_tensor("wd_t", [D, 1664], BF16, kind="ExternalInput").ap()
    t["wqu_t"] = nc.dram_tensor("wqu_t", [DQ, NH * HD], DT, kind="ExternalInput").ap()
    t["wabs_t"] = nc.dram_tensor(
        "wabs_t", [NH, 128, NKV, NOPE], DT, kind="ExternalInput"
    ).ap()
    t["wv_t"] = nc.dram_tensor(
        "wv_t", [128, NKV, NH * VD], DT, kind="ExternalInput"
    ).ap()
    t["wout_t"] = nc.dram_tensor(
        "wout_t", [NH * VD, D], BF16, kind="ExternalInput"
    ).ap()
    t["cosf"] = nc.dram_tensor("cosf", [128, L], F32, kind="ExternalInput").ap()
    t["sinf"] = nc.dram_tensor("sinf", [128, L], F32, kind="ExternalInput").ap()
    t["outT"] = nc.dram_tensor("outT", [D, L], BF16, kind="ExternalOutput").ap()

    with tile.TileContext(nc) as tc:
        _emit(tc, t)
    nc.compile()
    return nc


def _evict(nc, i, dst, src):
    """PSUM -> SBUF eviction, alternating engines to balance load."""
    if i % 2 == 0:
        nc.scalar.copy(dst, src)
    else:
        nc.vector.tensor_copy(dst, src)


def _rms_stats(tc, ps_d, prow, psq, ones_col_r, eps_t, lat, nlb, dim, r_row):
    """Compute per-token reciprocal RMS of `lat` ([128, nlb, L]) into
    r_row ([1, L], fp32).  No scaling is applied here — the scale is
    folded into downstream evictions so matmuls never wait on it."""
    nc = tc.nc
    for tk in range(NTOK):
        ts = slice(tk * TOK, (tk + 1) * TOK)
        ps_ssq = ps_d.tile([1, TOK], F32, tag="d")
        for lb in range(nlb):
            sq = psq.tile([128, TOK], DT, tag="sq")
            sl = lat[:, lb, ts]
            nc.vector.tensor_mul(sq, sl, sl)
            nc.tensor.matmul(
                ps_ssq, ones_col_r, sq, start=(lb == 0), stop=(lb == nlb - 1)
            )
        rt = prow.tile([1, TOK], F32, tag="rt")
        nc.scalar.activation(
            rt, ps_ssq, mybir.ActivationFunctionType.Sqrt,
            bias=eps_t, scale=1.0 / dim,
        )
        nc.vector.reciprocal_approx_fast(out=r_row[:, ts], in_=rt)


def _emit(tc, t):
    nc = tc.nc
    with ExitStack() as c0:
        c0.enter_context(
            nc.allow_low_precision(reason="fp32r/bf16 rounding is intentional")
        )
        from concourse import library_config

        nc.gpsimd.load_library(library_config.attnmlp)

        glob = c0.enter_context(tc.tile_pool(name="glob", bufs=1))
        ps_mm = c0.enter_context(tc.tile_pool(name="ps_mm", bufs=4, space="PSUM"))
        ps_v = c0.enter_context(tc.tile_pool(name="ps_v", bufs=2, space="PSUM"))
        ps_d = c0.enter_context(tc.tile_pool(name="ps_d", bufs=2, space="PSUM"))

        # ---- constants -----------------------------------------------
        ones_f32 = glob.tile([128, 128], F32, tag="ones32")
        nc.vector.memset(ones_f32, 1.0)
        ones_col_r = glob.tile([128, 1], DT, tag="onesr_c")
        nc.vector.tensor_copy(ones_col_r, ones_f32[:, :1])
        ones_col_b = glob.tile([128, 1], BF16, tag="onesb_c")
        nc.vector.tensor_copy(ones_col_b, ones_f32[:, :1])
        eps_t = glob.tile([1, 1], F32, tag="eps")
        nc.vector.memset(eps_t, EPS)
        k_roped = glob.tile([128, L], BF16, tag="kroped")
        v_all = glob.tile([128, NKB, NH * VD], BF16, tag="vall")

        with ExitStack() as cL:
            pL = cL.enter_context(tc.tile_pool(name="pL", bufs=1))
            cosf = pL.tile([128, L], F32, tag="cosf")
            nc.sync.dma_start(out=cosf, in_=t["cosf"])
            sinf = pL.tile([128, L], F32, tag="sinf")
            nc.sync.dma_start(out=sinf, in_=t["sinf"])
            kv_lat = pL.tile([128, NKV, L], DT, tag="kvlat")
            q_lat = pL.tile([128, NLAT, L], DT, tag="qlat")
            rkv_row = pL.tile([1, L], F32, tag="rkvrow")
            rq_row = pL.tile([1, L], F32, tag="rqrow")
            rkv_b = pL.tile([128, L], F32, tag="rkvb")
            rq_b = pL.tile([128, L], F32, tag="rqb")

            # ---- phase X: down projections (x in two d-halves) -------
            with ExitStack() as cX:
                px = cX.enter_context(tc.tile_pool(name="px", bufs=1))
                pwd = cX.enter_context(tc.tile_pool(name="pwd", bufs=3))
                prow = cX.enter_context(tc.tile_pool(name="prow", bufs=1))
                psq = cX.enter_context(tc.tile_pool(name="psq", bufs=3))
                pkr = cX.enter_context(tc.tile_pool(name="pkr", bufs=1))

                x_r = t["x_t"].rearrange("(b p) t -> p b t", p=128)
                wd_r = t["wd_t"].rearrange("(b p) m -> p b m", p=128)
                kr_pair = pkr.tile([128, 2, L], BF16, tag="krpair")

                warm_f = pkr.tile([128, TOK], F32, tag="warmf")
                nc.vector.memset(warm_f, 0.0)
                warm = pkr.tile([128, TOK], DT, tag="warm")
                nc.vector.tensor_copy(warm, warm_f)
                for _ in range(56):
                    ps = ps_mm.tile([128, TOK], F32, tag="mm")
                    nc.tensor.matmul(ps, warm[:, :128], warm)

                x_sb = px.tile([128, ND, L], BF16, tag="x")
                for ch in range(8):
                    nc.sync.dma_start(
                        out=x_sb[:, ch * 2 : ch * 2 + 2, :],
                        in_=x_r[:, ch * 2 : ch * 2 + 2, :],
                    )
                for ob in OB_ORDER:
                    cw = 64 if ob == 12 else 128
                    wd = pwd.tile([128, ND, 128], BF16, tag="wd")
                    nc.sync.dma_start(
                        out=wd[:, :, :cw],
                        in_=wd_r[:, :, ob * 128 : ob * 128 + cw],
                    )
                    pss = [
                        ps_mm.tile([128, TOK], F32, tag="mm",
                                   name=f"psd{tk}")
                        for tk in range(NTOK)
                    ]
                    for db in range(ND):
                        for tk in range(NTOK):
                            ts = slice(tk * TOK, (tk + 1) * TOK)
                            nc.tensor.matmul(
                                pss[tk][:cw], wd[:, db, :cw], x_sb[:, db, ts],
                                start=(db == 0), stop=(db == ND - 1),
                            )
                    for tk in range(NTOK):
                        ts = slice(tk * TOK, (tk + 1) * TOK)
                        if ob < 8:
                            dst = q_lat[:, ob, ts]
                        elif ob < 12:
                            dst = kv_lat[:, ob - 8, ts]
                        else:
                            dst = kr_pair[:64, 0, ts]
                        _evict(nc, ob + tk, dst, pss[tk][:cw])

                    if ob == 12:
                        # kv RMS stats + k rope (overlaps q blocks)
                        _rms_stats(tc, ps_d, prow, psq, ones_col_r,
                                   eps_t, kv_lat, NKV, KVR, rkv_row)
                        nc.gpsimd.partition_broadcast(rkv_b, rkv_row)
                        for lb in range(NKV):
                            nc.vector.tensor_mul(
                                kv_lat[:, lb, :], kv_lat[:, lb, :], rkv_b
                            )
                        nc.sync.dma_start(
                            out=kr_pair[0:32, 1, :], in_=kr_pair[32:64, 0, :]
                        )
                        nc.sync.dma_start(
                            out=kr_pair[32:64, 1, :], in_=kr_pair[0:32, 0, :]
                        )
                        nc.vector.tensor_mul(
                            k_roped[0:64], kr_pair[0:64, 0, :], cosf[0:64]
                        )
                        nc.vector.tensor_mul(
                            kr_pair[0:64, 0, :], kr_pair[0:64, 1, :],
                            sinf[0:64],
                        )
                        nc.vector.tensor_add(
                            k_roped[0:64], k_roped[0:64], kr_pair[0:64, 0, :]
                        )
                        nc.sync.dma_start(
                            out=k_roped[64:128], in_=k_roped[0:64]
                        )

                # q RMS stats (resolve while kv_abs/V matmuls run)
                _rms_stats(tc, ps_d, prow, psq, ones_col_r,
                           eps_t, q_lat, NLAT, DQ, rq_row)
                nc.gpsimd.partition_broadcast(rq_b, rq_row)

            # ---- phases P/Q/A under attention-lived pools ------------
            pwo = cL.enter_context(tc.tile_pool(name="pwo", bufs=4))
            pvn = cL.enter_context(tc.tile_pool(name="pvn", bufs=1))
            vn = pvn.tile([128, NH, L], BF16, tag="vn")
            wout_r = t["wout_t"].rearrange("(b p) m -> p b m", p=128)
            wouts = [None] * 16

            def fetch_wout(c):
                wouts[c] = pwo.tile([128, NH, 128], BF16, tag="wout", name=f"wout{c}")
                nc.sync.dma_start(
                    out=wouts[c], in_=wout_r[:, :, c * 128 : (c + 1) * 128]
                )

            with ExitStack() as cM:
                pM = cM.enter_context(tc.tile_pool(name="pM", bufs=1))
                kv_abs = pM.tile([128, NH, L], DT, tag="kvabs")
                qT_nope = pM.tile([128, NH, L], DT, tag="qnope")
                q_roped = pM.tile([128, NH // 2, L], BF16, tag="qroped")

                # ---- phase P: kv_abs + V precompute ------------------
                with ExitStack() as cP:
                    pw = cP.enter_context(tc.tile_pool(name="pw", bufs=2))
                    for h in range(NH):
                        wabs = pw.tile([128, NKV, NOPE], DT, tag="wabs")
                        nc.sync.dma_start(out=wabs, in_=t["wabs_t"][h])
                        pss = [
                            ps_mm.tile([128, TOK], F32, tag="mm",
                                       name=f"psp{tk}")
                            for tk in range(NTOK)
                        ]
                        for lb in range(NKV):
                            for tk in range(NTOK):
                                ts = slice(tk * TOK, (tk + 1) * TOK)
                                nc.tensor.matmul(
                                    pss[tk], wabs[:, lb], kv_lat[:, lb, ts],
                                    start=(lb == 0), stop=(lb == NKV - 1),
                                )
                        for tk in range(NTOK):
                            ts = slice(tk * TOK, (tk + 1) * TOK)
                            _evict(nc, h + tk, kv_abs[:, h, ts], pss[tk])

                    for qc in range(2):
                        hv = slice(qc * 512, (qc + 1) * 512)
                        wv = pw.tile([128, NKV, 512], BF16, tag="wv")
                        nc.sync.dma_start(out=wv, in_=t["wv_t"][:, :, hv])
                        for kp in range(NKB // 2):
                            pss = [
                                ps_mm.tile([128, 512], F32, tag="mm",
                                           name=f"psv{ki}")
                                for ki in range(2)
                            ]
                            for lb in range(NKV):
                                for ki in range(2):
                                    kb = kp * 2 + ki
                                    ks = slice(kb * KB, (kb + 1) * KB)
                                    nc.tensor.matmul(
                                        pss[ki], kv_lat[:, lb, ks],
                                        wv[:, lb, :],
                                        start=(lb == 0), stop=(lb == NKV - 1),
                                    )
                            for ki in range(2):
                                kb = kp * 2 + ki
                                _evict(nc, qc + kp + ki, v_all[:, kb, hv],
                                       pss[ki])

                # ---- phase Q: q up-projection + q rope ---------------
                with ExitStack() as cQ:
                    pqu = cQ.enter_context(tc.tile_pool(name="pqu", bufs=4))
                    ppair = cQ.enter_context(tc.tile_pool(name="ppair", bufs=1))
                    wqu_r = t["wqu_t"].rearrange("(b p) m -> p b m", p=128)
                    for p in range(NH // 2):
                        q_pair = ppair.tile([128, 2, L], BF16, tag="pair")
                        for piece in range(3):
                            col0 = p * 384 + piece * 128
                            wqu = pqu.tile([128, NLAT, 128], DT, tag="wqu")
                            nc.sync.dma_start(
                                out=wqu, in_=wqu_r[:, :, col0 : col0 + 128]
                            )
                            pss = [
                                ps_mm.tile([128, TOK], F32, tag="mm",
                                           name=f"psq{tk}")
                                for tk in range(NTOK)
                            ]
                            for lb in range(NLAT):
                                for tk in range(NTOK):
                                    ts = slice(tk * TOK, (tk + 1) * TOK)
                                    nc.tensor.matmul(
                                        pss[tk], wqu[:, lb], q_lat[:, lb, ts],
                                        start=(lb == 0), stop=(lb == NLAT - 1),
                                    )
                            for tk in range(NTOK):
                                ts = slice(tk * TOK, (tk + 1) * TOK)
                                if piece < 2:
                                    nc.vector.tensor_mul(
                                        qT_nope[:, 2 * p + piece, ts],
                                        pss[tk], rq_b[:, ts],
                                    )
                                else:
                                    nc.vector.tensor_mul(
                                        q_pair[:, 0, ts], pss[tk], rq_b[:, ts]
                                    )
                        nc.sync.dma_start(
                            out=q_pair[0:32, 1, :], in_=q_pair[32:64, 0, :]
                        )
                        nc.sync.dma_start(
                            out=q_pair[32:64, 1, :], in_=q_pair[0:32, 0, :]
                        )
                        nc.sync.dma_start(
                            out=q_pair[64:96, 1, :], in_=q_pair[96:128, 0, :]
                        )
                        nc.sync.dma_start(
                            out=q_pair[96:128, 1, :], in_=q_pair[64:96, 0, :]
                        )
                        nc.vector.tensor_mul(
                            q_roped[:, p, :], q_pair[:, 0, :], cosf
                        )
                        nc.vector.tensor_mul(
                            q_pair[:, 0, :], q_pair[:, 1, :], sinf
                        )
                        nc.vector.tensor_add(
                            q_roped[:, p, :], q_roped[:, p, :], q_pair[:, 0, :]
                        )

                # ---- phase A: attention ------------------------------
                with ExitStack() as cA:
                    pe = cA.enter_context(tc.tile_pool(name="pe", bufs=5))
                    prd = cA.enter_context(tc.tile_pool(name="prd", bufs=2))

                    for h in range(NH):
                        hb = (h % 2) * 64
                        pr = h // 2
                        hv = slice(h * VD, (h + 1) * VD)
                        ps_vt = {}
                        ps_dt = {}
                        for qb in range(NTOK):
                            ps_vt[qb] = ps_v.tile([128, TOK], F32, tag="v", name=f"psvt{qb}")
                            ps_dt[qb] = ps_d.tile([1, TOK], F32, tag="d", name=f"psdt{qb}")
                        for kb in range(NKB):
                            k0 = kb * KB
                            ks = slice(k0, k0 + KB)
                            us = UNITS[kb]
                            sts = [
                                ps_mm.tile([128, TOK], F32, tag="mm",
                                           name=f"pss{ui}")
                                for ui in range(len(us))
                            ]
                            for (qb, qs, f, dg), st in zip(us, sts):
                                nc.tensor.matmul(
                                    st[:, :f], kv_abs[:, h, ks],
                                    qT_nope[:, h, qs : qs + f],
                                    start=True, stop=False,
                                )
                            for (qb, qs, f, dg), st in zip(us, sts):
                                nc.tensor.matmul(
                                    st[:, :f], k_roped[hb : hb + 64, ks],
                                    q_roped[hb : hb + 64, pr, qs : qs + f],
                                    start=False, stop=True,
                                )
                            ets = []
                            for (qb, qs, f, dg), st in zip(us, sts):
                                e_t = pe.tile([128, TOK], BF16, tag="e")
                                nc.scalar.activation(
                                    e_t[:, :f], st[:, :f],
                                    mybir.ActivationFunctionType.Exp,
                                    scale=SCALE,
                                )
                                if dg:
                                    nc.gpsimd.affine_select(
                                        out=e_t[:, :f], in_=e_t[:, :f],
                                        pattern=[[1, f]],
                                        compare_op=mybir.AluOpType.is_ge,
                                        fill=0.0,
                                        base=qs - k0,
                                        channel_multiplier=-1,
                                    )
                                ets.append(e_t)
                            for (qb, qs, f, dg), e_t in zip(us, ets):
                                lo = qs - qb * TOK
                                nc.tensor.matmul(
                                    ps_dt[qb][:, lo : lo + f], ones_col_b,
                                    e_t[:, :f],
                                    start=(kb == FIRST_KB[qb]),
                                    stop=(kb == LAST_KB[qb]),
                                )
                            for (qb, qs, f, dg), e_t in zip(us, ets):
                                lo = qs - qb * TOK
                                nc.tensor.matmul(
                                    ps_vt[qb][:, lo : lo + f],
                                    v_all[:, kb, hv], e_t[:, :f],
                                    start=(kb == FIRST_KB[qb]),
                                    stop=(kb == LAST_KB[qb]),
                                )
                            for qb in range(NTOK):
                                if kb == LAST_KB[qb]:
                                    ts = slice(qb * TOK, (qb + 1) * TOK)
                                    rd = prd.tile([1, TOK], F32, tag="rd")
                                    nc.vector.reciprocal_approx_fast(
                                        out=rd, in_=ps_dt[qb]
                                    )
                                    rb = prd.tile([128, TOK], F32, tag="rb")
                                    nc.gpsimd.partition_broadcast(rb, rd)
                                    nc.vector.tensor_mul(
                                        vn[:, h, ts], ps_vt[qb], rb
                                    )
                        if h == 5:
                            fetch_wout(0)
                        if h == 6:
                            fetch_wout(1)
                            fetch_wout(2)

            # ---- phase O: output projection (cM closed) --------------
            with ExitStack() as cO:
                po = cO.enter_context(tc.tile_pool(name="po", bufs=4))
                for c in range(16):
                    if wouts[c] is None:
                        fetch_wout(c)
                    row = c * 128
                    pss = [
                        ps_mm.tile([128, TOK], F32, tag="mm",
                                   name=f"pso{tk}")
                        for tk in range(NTOK)
                    ]
                    for hbk in range(NH):
                        for tk in range(NTOK):
                            ts = slice(tk * TOK, (tk + 1) * TOK)
                            nc.tensor.matmul(
                                pss[tk], wouts[c][:, hbk, :], vn[:, hbk, ts],
                                start=(hbk == 0), stop=(hbk == NH - 1),
                            )
                    for tk in range(NTOK):
                        ts = slice(tk * TOK, (tk + 1) * TOK)
                        o_t = po.tile([128, TOK], BF16, tag="o")
                        _evict(nc, c + tk, o_t, pss[tk])
                        nc.sync.dma_start(
                            out=t["outT"][row : row + 128, ts], in_=o_t
                        )


# ======================================================================
# host side
# ======================================================================

_NC_CACHE = {}


def _get_nc():
    if "nc" not in _NC_CACHE:
        _NC_CACHE["nc"] = build_nc()
    return _NC_CACHE["nc"]


def _prep_shared(inputs):
    wq_down = np.asarray(inputs["Wq_down"], np.float32)
    wq_up = np.asarray(inputs["Wq_up"], np.float32)
    wkv_down = np.asarray(inputs["Wkv_down"], np.float32)
    wkv_up = np.asarray(inputs["Wkv_up"], np.float32)
    wout = np.asarray(inputs["Wout"], np.float32)
    rms_q_w = np.asarray(inputs["rms_q_w"], np.float32)
    rms_kv_w = np.asarray(inputs["rms_kv_w"], np.float32)
    freq = np.asarray(inputs["freq_cis"], np.float32)  # [L, 32, 2]

    # split re/im layout for all rope dims: re parts first, then im parts
    rope_perm = np.concatenate(
        [np.arange(0, ROPE, 2), np.arange(1, ROPE, 2)]
    )  # [64]

    # combined down-proj: q latent | kv latent | k-rope (re/im split), pad
    wd = np.zeros((1664, D), np.float32)
    wd[:DQ] = wq_down
    wd[DQ : DQ + KVR] = wkv_down[:KVR]
    wd[DQ + KVR : DQ + KVR + ROPE] = wkv_down[KVR:][rope_perm]
    wd_t = np.ascontiguousarray(wd.T).astype(ml_dtypes.bfloat16)  # [D, 1664]

    # rope tables (dim-major, split re/im, duplicated partition halves)
    cos = freq[:, :, 0].T  # [32, L]
    sin = freq[:, :, 1].T
    cosf64 = np.vstack([cos, cos])  # [64, L]
    sinf64 = np.vstack([-sin, sin])
    cosf = np.ascontiguousarray(np.vstack([cosf64, cosf64]))  # [128, L]
    sinf = np.ascontiguousarray(np.vstack([sinf64, sinf64]))

    wq_up3 = (wq_up * rms_q_w[None, :]).reshape(H, HD, DQ)
    wq_nope = wq_up3[:, :NOPE, :]                      # [H, 128, DQ]
    wq_rope = wq_up3[:, NOPE:, :][:, rope_perm, :]     # [H, 64, DQ]
    wkv_up3 = wkv_up.reshape(H, NOPE + VD, KVR)
    wout3 = wout.reshape(D, H, VD)

    per_g = []
    for g in range(2):
        hs = list(range(g * NH, (g + 1) * NH))
        # q up: per pair [nope(2p) | nope(2p+1) | rope(2p)+rope(2p+1)]
        cols = []
        for p in range(NH // 2):
            h0, h1 = hs[2 * p], hs[2 * p + 1]
            cols.append(wq_nope[h0])
            cols.append(wq_nope[h1])
            cols.append(wq_rope[h0])
            cols.append(wq_rope[h1])
        wqu_t = np.ascontiguousarray(
            np.concatenate(cols, axis=0).T
        )  # [DQ, 1536]

        wabs = wkv_up3[hs, :NOPE, :] * rms_kv_w[None, None, :]  # [8,128,512]
        # per head: [KVR, NOPE] -> [128, 4, 128]
        wabs_t = np.ascontiguousarray(
            wabs.transpose(0, 2, 1).reshape(NH, NKV, 128, NOPE)
            .transpose(0, 2, 1, 3)
        )  # [8, 128, 4, 128]

        wv = wkv_up3[hs, NOPE:, :] * rms_kv_w[None, None, :]  # [8, 128, 512]
        # [KVR, NH*VD] -> [128, 4, 1024]
        wv_t = np.ascontiguousarray(
            wv.transpose(2, 0, 1).reshape(NKV, 128, NH * VD)
            .transpose(1, 0, 2)
        )  # [128, 4, 1024]

        wout_t = np.ascontiguousarray(
            wout3[:, hs, :].transpose(1, 2, 0).reshape(NH * VD, D)
        ).astype(ml_dtypes.bfloat16)  # [1024, 2048]
        per_g.append(
            {
                "wd_t": wd_t,
                "wqu_t": wqu_t,
                "wabs_t": wabs_t,
                "wv_t": wv_t,
                "wout_t": wout_t,
                "cosf": cosf,
                "sinf": sinf,
            }
        )
    return per_g


def make_in_maps(inputs):
    x = np.asarray(inputs["x"], np.float32)
    per_g = _prep_shared(inputs)
    in_maps = []
    for c in range(N_CORES):
        b, g = c // 2, c % 2
        m = dict(per_g[g])
        m["x_t"] = np.ascontiguousarray(x[b].T).astype(ml_dtypes.bfloat16)
        in_maps.append(m)
    return in_maps


def kernel(**inputs):
    nc = _get_nc()
    in_maps = make_in_maps(inputs)
    res = bass_utils.run_bass_kernel_spmd(
        nc, in_maps, core_ids=list(range(N_CORES))
    ).results
    out = np.empty((B, L, D), np.float32)
    for b in range(B):
        out[b] = (res[2 * b]["outT"].astype(np.float32)
                  + res[2 * b + 1]["outT"].astype(np.float32)).T
    return out
